# revision 12
# baseline (speedup 1.0000x reference)
"""Trainium2 Bass kernel for nn_Decoder_65498251264356.

Pointer-generator decoder step: embedding + LSTM cell + Bahdanau attention
(with coverage) + pointer-gate + vocab softmax + pointer scatter-mix.

Sharding: data-parallel over batch B=128 across 8 cores (16 rows each) for
the LSTM/attention front (launch A); the vocab projection is
tensor-parallel over V=50000 (6250 columns per core, launch B) using the
s2/p_gen rows gathered on the host between launches; the vocab softmax
normalizer is an 8-way partial-sum exchange through the host, applied
on-device in launch C. The final pointer scatter-add is applied on the
host during unsharding, using device-computed attn and p_gen.
(Device collectives compile but fail to load under this axon terminal, so
cross-core exchanges ride the host launch boundaries instead.)
"""
import sys

sys.path.insert(0, "/opt/trn_rl_repo")

import numpy as np
import ml_dtypes

import concourse.bass as bass
import concourse.mybir as mybir
import concourse.tile as tile
from concourse.masks import make_identity
from concourse.vector_clock import ScopedClock

dt = mybir.dt
AF = mybir.ActivationFunctionType
ALU = mybir.AluOpType

B, T, H, E, V = 128, 400, 512, 128, 50000
NCORES = 8
BL = B // NCORES        # 16 batch rows per core
VL = V // NCORES        # 6250 vocab cols per core
H2 = 2 * H              # 1024
EPS = 1e-12
BF = dt.bfloat16
F32 = dt.float32

NCH = [(i * 512, min(512, VL - i * 512)) for i in range((VL + 511) // 512)]


# ---------------------------------------------------------------------------
# walrus in this container rejects >1 sem wait per instruction; split the
# Tile tail-drain's aggregated waits onto single-wait NOPs.
def _patched_drain_and_barrier(self, tick_clock, wait_clock):
    nc = self.nc
    carrier = nc.sync.nop(nofuse=True)
    wait_clock.add_sem_waits(carrier.ins, ScopedClock({None: tick_clock.global_clock}))
    si = carrier.ins.sync_info
    waits = list(si.on_wait or []) if si else []
    if len(waits) > 1:
        carrier.ins.sync_info = mybir.SyncInfo(
            on_wait=waits[:1], on_update=list(si.on_update or [])
        )
        for w in waits[1:]:
            n = nc.sync.nop(nofuse=True)
            n.ins.sync_info = mybir.SyncInfo(on_wait=[w], on_update=[])
    nc.sync.drain()
    nc.all_engine_barrier()
    assert self.sems is not None
    popped = nc._tile_sem_poison_stack.pop()
    assert popped is self._sem_poison
    nc.clear_and_free_semaphores(list(self.sems.allocated().values()))
    nc.all_engine_barrier()


tile.TileContext._drain_and_barrier = _patched_drain_and_barrier

SPLIT_WAITS = True
_wsplit_ctr = [0]


def _split_multi_waits(nc):
    if not SPLIT_WAITS:
        return
    """Same walrus limit, applied globally: extra waits move onto
    single-wait NOPs inserted just before the instruction, same engine."""
    for f in nc.m.functions:
        for bb in f.blocks:
            il = bb.instructions
            i = 0
            while i < len(il):
                inst = il[i]
                si = inst.sync_info
                waits = list(si.on_wait) if si and si.on_wait else []
                if len(waits) > 1:
                    for w in waits[:-1]:
                        _wsplit_ctr[0] += 1
                        nop = mybir.InstNoOp(
                            name=f"I-wsplit-{_wsplit_ctr[0]}",
                            engine=inst.engine,
                            sync_info=mybir.SyncInfo(on_wait=[w], on_update=[]),
                        )
                        il.insert(i, nop)
                        i += 1
                    inst.sync_info = mybir.SyncInfo(
                        on_wait=[waits[-1]], on_update=list(si.on_update or [])
                    )
                i += 1
# ---------------------------------------------------------------------------


def build_program_a(use_cov: bool):
    """LSTM + attention + p_gen + s2, data-parallel over 16 batch rows."""
    nc = bass.Bass()

    xinT = nc.dram_tensor("xinT", [9 * E, BL], F32, kind="ExternalInput")   # [c_t_1; emb].T
    h0T = nc.dram_tensor("h0T", [H, BL], F32, kind="ExternalInput")
    c0T = nc.dram_tensor("c0T", [H, BL], F32, kind="ExternalInput")
    encfT = nc.dram_tensor("encfT", [BL, H2, T], BF, kind="ExternalInput")  # feature-major
    enco = nc.dram_tensor("enco", [BL, T, H2], BF, kind="ExternalInput")    # natural
    mask = nc.dram_tensor("mask", [BL, T], F32, kind="ExternalInput")
    cov = nc.dram_tensor("cov", [BL, T], F32, kind="ExternalInput")
    Wxc = nc.dram_tensor("Wxc", [9 * E, E], F32, kind="ExternalInput")      # W_xc.T
    bxc = nc.dram_tensor("bxc", [E, 1], F32, kind="ExternalInput")
    Wih = nc.dram_tensor("Wih", [E, 4 * H], F32, kind="ExternalInput")      # W_ih.T
    Whh = nc.dram_tensor("Whh", [H, 4 * H], F32, kind="ExternalInput")      # W_hh.T
    bih2 = nc.dram_tensor("bih2", [128, 16], F32, kind="ExternalInput")
    bhh2 = nc.dram_tensor("bhh2", [128, 16], F32, kind="ExternalInput")
    Wdp = nc.dram_tensor("Wdp", [H2, H2], F32, kind="ExternalInput")        # W_dp.T
    bdp2 = nc.dram_tensor("bdp2", [128, 8], F32, kind="ExternalInput")
    vw2 = nc.dram_tensor("vw2", [128, 8], BF, kind="ExternalInput")         # v chunks
    wcrow = nc.dram_tensor("wcrow", [1, H2], BF, kind="ExternalInput")      # W_c row
    covrow = nc.dram_tensor("covrow", [1, BL * T], BF, kind="ExternalInput")
    Wpg = nc.dram_tensor("Wpg", [128, 17], F32, kind="ExternalInput")       # W_pg.T chunks
    bpg = nc.dram_tensor("bpg", [1, 1], F32, kind="ExternalInput")
    Wo1 = nc.dram_tensor("Wo1", [3 * H, H], F32, kind="ExternalInput")      # W_o1.T
    bo12 = nc.dram_tensor("bo12", [128, 4], F32, kind="ExternalInput")

    h_out = nc.dram_tensor("h_out", [H, BL], F32, kind="ExternalOutput")
    c_out = nc.dram_tensor("c_out", [H, BL], F32, kind="ExternalOutput")
    ct_out = nc.dram_tensor("ct_out", [H2, BL], F32, kind="ExternalOutput")
    attn_out = nc.dram_tensor("attn_out", [BL, T], F32, kind="ExternalOutput")
    pg_out = nc.dram_tensor("pg_out", [BL, 1], F32, kind="ExternalOutput")
    covn_out = nc.dram_tensor("covn_out", [BL, T], F32, kind="ExternalOutput")
    s2_out = nc.dram_tensor("s2_out", [H, BL], F32, kind="ExternalOutput")

    with tile.TileContext(nc) as tc:
        with (
            tc.tile_pool(name="wp", bufs=1) as wp,
            tc.tile_pool(name="fp", bufs=1) as fp,
            tc.tile_pool(name="ap", bufs=3) as ap,
            tc.tile_pool(name="pss", bufs=2, space="PSUM") as pss,
            tc.tile_pool(name="pssc", bufs=2, space="PSUM") as pssc,
            tc.tile_pool(name="psct", bufs=1, space="PSUM") as psct,
        ):
            # ---- persistent small tiles ----
            vw_t = wp.tile([128, 8], BF)
            nc.sync.dma_start(out=vw_t[:], in_=vw2[:])
            bxc_t = wp.tile([128, 1], F32)
            nc.sync.dma_start(out=bxc_t[:], in_=bxc[:])
            bih_t = wp.tile([128, 16], F32)
            nc.sync.dma_start(out=bih_t[:], in_=bih2[:])
            bhh_t = wp.tile([128, 16], F32)
            nc.sync.dma_start(out=bhh_t[:], in_=bhh2[:])
            bdp_t = wp.tile([128, 8], F32)
            nc.sync.dma_start(out=bdp_t[:], in_=bdp2[:])
            bo1_t = wp.tile([128, 4], F32)
            nc.sync.dma_start(out=bo1_t[:], in_=bo12[:])
            bpg_t = wp.tile([1, 1], F32)
            nc.sync.dma_start(out=bpg_t[:], in_=bpg[:])
            wpg_t = wp.tile([128, 17], F32)
            nc.sync.dma_start(out=wpg_t[:], in_=Wpg[:])
            mask_t = wp.tile([BL, T], F32)
            nc.sync.dma_start(out=mask_t[:], in_=mask[:])
            cov_t = wp.tile([BL, T], F32)
            nc.sync.dma_start(out=cov_t[:], in_=cov[:])
            if use_cov:
                wcrow_t = wp.tile([1, H2], BF)
                nc.sync.dma_start(out=wcrow_t[:], in_=wcrow[:])
                covrow_t = wp.tile([1, BL * T], BF)
                nc.sync.dma_start(out=covrow_t[:], in_=covrow[:])
            ident = wp.tile([128, 128], F32)
            make_identity(nc, ident[:])

            # combined LSTM gate bias, plus halved for sigmoid-via-tanh:
            # sigmoid(x) = 0.5*tanh(0.5*x) + 0.5
            bg_t = wp.tile([128, 16], F32)
            nc.vector.tensor_tensor(out=bg_t[:], in0=bih_t[:], in1=bhh_t[:], op=ALU.add)
            bgh_t = wp.tile([128, 16], F32)
            nc.vector.tensor_scalar_mul(out=bgh_t[:], in0=bg_t[:], scalar1=0.5)
            bpgh_t = wp.tile([1, 1], F32)
            nc.vector.tensor_scalar_mul(out=bpgh_t[:], in0=bpg_t[:], scalar1=0.5)

            # ---- big front weights (recycled slots) ----
            wxc_t = fp.tile([128, 9, E], F32, tag="wxc")
            nc.sync.dma_start(out=wxc_t[:], in_=Wxc[:].rearrange("(c p) m -> p c m", p=128))
            wih_t = fp.tile([128, 4 * H], F32, tag="wih")
            nc.sync.dma_start(out=wih_t[:], in_=Wih[:])
            whh_t = fp.tile([128, 4, 4 * H], F32, tag="whh")
            nc.sync.dma_start(out=whh_t[:], in_=Whh[:].rearrange("(c p) m -> p c m", p=128))
            wdp_t = fp.tile([128, 8, H2], F32, tag="wdp")
            nc.sync.dma_start(out=wdp_t[:], in_=Wdp[:].rearrange("(c p) m -> p c m", p=128))
            wo1_t = fp.tile([128, 12, H], F32, tag="wo1")
            nc.sync.dma_start(out=wo1_t[:], in_=Wo1[:].rearrange("(c p) m -> p c m", p=128))

            # ---- x = [c_t_1; emb] @ W_xc.T + b ----
            xin_t = fp.tile([128, 9, BL], F32, tag="xin")
            nc.sync.dma_start(out=xin_t[:], in_=xinT[:].rearrange("(c p) b -> p c b", p=128))
            h0_t = fp.tile([128, 4, BL], F32, tag="h0")
            nc.sync.dma_start(out=h0_t[:], in_=h0T[:].rearrange("(c p) b -> p c b", p=128))
            c0_t = fp.tile([128, 4, BL], F32, tag="c0")
            nc.sync.dma_start(out=c0_t[:], in_=c0T[:].rearrange("(c p) b -> p c b", p=128))

            ps_x = pss.tile([128, BL], F32, space="PSUM", tag="ps_small")
            for c in range(9):
                nc.tensor.matmul(out=ps_x[:], lhsT=wxc_t[:, c, :], rhs=xin_t[:, c, :],
                                 start=(c == 0), stop=(c == 8))
            xs = fp.tile([128, BL], F32, tag="xs")
            nc.vector.tensor_scalar_add(out=xs[:], in0=ps_x[:], scalar1=bxc_t[:, 0:1])

            # ---- LSTM gates (order i,f,g,o) ----
            i_s = fp.tile([128, 4, BL], F32, tag="i_s")
            f_s = fp.tile([128, 4, BL], F32, tag="f_s")
            g_s = fp.tile([128, 4, BL], F32, tag="g_s")
            o_s = fp.tile([128, 4, BL], F32, tag="o_s")
            gate_dst = [i_s, f_s, g_s, o_s]
            for mc in range(16):
                ps_g = pss.tile([128, BL], F32, space="PSUM", tag="ps_small")
                nc.tensor.matmul(out=ps_g[:], lhsT=wih_t[:, mc * 128:(mc + 1) * 128],
                                 rhs=xs[:], start=True, stop=False)
                for c in range(4):
                    nc.tensor.matmul(out=ps_g[:], lhsT=whh_t[:, c, mc * 128:(mc + 1) * 128],
                                     rhs=h0_t[:, c, :], start=False, stop=(c == 3))
                dst = gate_dst[mc // 4][:, mc % 4, :]
                if mc // 4 == 2:  # g -> tanh(x + b)
                    nc.scalar.activation(out=dst, in_=ps_g[:], func=AF.Tanh,
                                         bias=bg_t[:, mc:mc + 1], scale=1.0)
                else:  # i,f,o -> sigmoid via tanh
                    nc.scalar.activation(out=dst, in_=ps_g[:], func=AF.Tanh,
                                         bias=bgh_t[:, mc:mc + 1], scale=0.5)
            for gidx in (0, 1, 3):
                g = gate_dst[gidx]
                nc.vector.tensor_scalar(out=g[:], in0=g[:], scalar1=0.5, scalar2=0.5,
                                        op0=ALU.mult, op1=ALU.add)

            # ---- c, h ----
            cT_s = fp.tile([128, 4, BL], F32, tag="cT")
            hT_s = fp.tile([128, 4, BL], F32, tag="hT")
            tnc = fp.tile([128, 4, BL], F32, tag="tnc")
            for c in range(4):
                t1 = fp.tile([128, BL], F32, tag="lstm_t1")
                nc.vector.tensor_tensor(out=t1[:], in0=f_s[:, c, :], in1=c0_t[:, c, :], op=ALU.mult)
                t2 = fp.tile([128, BL], F32, tag="lstm_t2")
                nc.vector.tensor_tensor(out=t2[:], in0=i_s[:, c, :], in1=g_s[:, c, :], op=ALU.mult)
                nc.vector.tensor_tensor(out=cT_s[:, c, :], in0=t1[:], in1=t2[:], op=ALU.add)
                nc.scalar.activation(out=tnc[:, c, :], in_=cT_s[:, c, :], func=AF.Tanh)
                nc.vector.tensor_tensor(out=hT_s[:, c, :], in0=o_s[:, c, :], in1=tnc[:, c, :], op=ALU.mult)

            nc.sync.dma_start(out=h_out[:].rearrange("(c p) b -> p c b", p=128), in_=hT_s[:])
            nc.sync.dma_start(out=c_out[:].rearrange("(c p) b -> p c b", p=128), in_=cT_s[:])

            # ---- dec_fea = s_t_hat @ W_dp.T + b_dp ----
            dec_t = fp.tile([128, 8, BL], F32, tag="dec")
            for mc in range(8):
                ps_d = pss.tile([128, BL], F32, space="PSUM", tag="ps_small")
                for c in range(8):
                    rhs = hT_s[:, c, :] if c < 4 else cT_s[:, c - 4, :]
                    nc.tensor.matmul(out=ps_d[:], lhsT=wdp_t[:, c, mc * 128:(mc + 1) * 128],
                                     rhs=rhs, start=(c == 0), stop=(c == 7))
                nc.vector.tensor_scalar_add(out=dec_t[:, mc, :], in0=ps_d[:],
                                            scalar1=bdp_t[:, mc:mc + 1])

            # ---- attention pass 1: scores -> softmax ----
            # (engines may only address partition strips at 0/32/64/96, so
            # per-b exp results land on partition 0 as column slices and are
            # redistributed to [BL, T] with one SBUF->SBUF DMA)
            expsc_row = fp.tile([1, BL * T], F32, tag="expsc_row")
            ssum_row = fp.tile([1, BL], F32, tag="ssum_row")
            for b in range(BL):
                eF = ap.tile([128, 8, T], BF, tag="eF")
                nc.sync.dma_start(out=eF[:], in_=encfT[b].rearrange("(c p) t -> p c t", p=128))
                eE = ap.tile([128, 8, T], BF, tag="eE")
                if use_cov:
                    for c in range(8):
                        ps_cov = pssc.tile([128, T], F32, space="PSUM", tag="ps_cov")
                        nc.tensor.matmul(out=ps_cov[:], lhsT=wcrow_t[:, c * 128:(c + 1) * 128],
                                         rhs=covrow_t[:, b * T:(b + 1) * T], start=True, stop=True)
                        t3 = ap.tile([128, T], F32, tag="covtmp")
                        nc.vector.tensor_scalar_add(out=t3[:], in0=eF[:, c, :],
                                                    scalar1=dec_t[:, c, b:b + 1])
                        nc.vector.tensor_tensor(out=eE[:, c, :], in0=t3[:], in1=ps_cov[:], op=ALU.add)
                else:
                    for c in range(8):
                        nc.vector.tensor_scalar_add(out=eE[:, c, :], in0=eF[:, c, :],
                                                    scalar1=dec_t[:, c, b:b + 1])
                nc.scalar.activation(out=eE[:], in_=eE[:], func=AF.Tanh)
                ps_sc = pssc.tile([1, T], F32, space="PSUM", tag="ps_sc")
                for c in range(8):
                    nc.tensor.matmul(out=ps_sc[:], lhsT=vw_t[:, c:c + 1], rhs=eE[:, c, :],
                                     start=(c == 0), stop=(c == 7))
                nc.scalar.activation(out=expsc_row[:, b * T:(b + 1) * T], in_=ps_sc[:],
                                     func=AF.Exp, accum_out=ssum_row[:, b:b + 1])

            expsc = fp.tile([BL, T], F32, tag="expsc")
            nc.sync.dma_start(out=expsc[:], in_=expsc_row[:].rearrange("x (b t) -> x b t", b=BL))
            ssum = fp.tile([BL, 1], F32, tag="ssum")
            nc.sync.dma_start(out=ssum[:], in_=ssum_row[:].rearrange("x (b o) -> x b o", b=BL))

            # softmax tail; equals attn_/(sum(attn_)+eps), attn_ = softmax*mask
            m1 = fp.tile([BL, T], F32, tag="m1")
            nc.vector.tensor_tensor(out=m1[:], in0=expsc[:], in1=mask_t[:], op=ALU.mult)
            s1 = fp.tile([BL, 1], F32, tag="s1")
            nc.vector.reduce_sum(out=s1[:], in_=m1[:], axis=mybir.AxisListType.X)
            den = fp.tile([BL, 1], F32, tag="den")
            nc.vector.tensor_scalar_mul(out=den[:], in0=ssum[:], scalar1=float(EPS))
            nc.vector.tensor_tensor(out=den[:], in0=den[:], in1=s1[:], op=ALU.add)
            rden = fp.tile([BL, 1], F32, tag="rden")
            nc.vector.reciprocal(out=rden[:], in_=den[:])
            attn_t = fp.tile([BL, T], F32, tag="attn")
            nc.vector.tensor_scalar_mul(out=attn_t[:], in0=m1[:], scalar1=rden[:, 0:1])
            nc.sync.dma_start(out=attn_out[:], in_=attn_t[:])
            covn_t = fp.tile([BL, T], F32, tag="covn")
            nc.vector.tensor_tensor(out=covn_t[:], in0=cov_t[:], in1=attn_t[:], op=ALU.add)
            nc.sync.dma_start(out=covn_out[:], in_=covn_t[:])

            # attn.T chunks for c_t matmuls (PE transpose), bf16.
            # T=400 wraps as 4 chunks of 100 partitions so enco[b] loads in
            # ONE DMA below (no ragged 16-row tail transfer).
            attnT_s = fp.tile([100, 4, BL], BF, tag="attnT")
            for q in range(4):
                lo = q * 100
                ps_tr = pss.tile([128, BL], F32, space="PSUM", tag="ps_small")
                nc.tensor.transpose(out=ps_tr[:100, :], in_=attn_t[:, lo:lo + 100],
                                    identity=ident[:BL, :BL])
                nc.vector.tensor_copy(out=attnT_s[:, q, :], in_=ps_tr[:100, :])

            # ---- attention pass 2: c_t ----
            ps_ct = psct.tile([128, 8 * BL], F32, space="PSUM")
            for b in range(BL):
                oT = ap.tile([100, 4, H2], BF, tag="oT")
                nc.sync.dma_start(out=oT[:], in_=enco[b].rearrange("(q p) f -> p q f", p=100))
                for fc in range(8):
                    col = fc * BL + b
                    for q in range(4):
                        nc.tensor.matmul(out=ps_ct[:, col:col + 1],
                                         lhsT=oT[:, q, fc * 128:(fc + 1) * 128],
                                         rhs=attnT_s[:, q, b:b + 1],
                                         start=(q == 0), stop=(q == 3))
            ct_s = fp.tile([128, 8, BL], F32, tag="ct")
            nc.vector.tensor_copy(out=ct_s[:], in_=ps_ct[:].rearrange("p (fc b) -> p fc b", fc=8))
            nc.sync.dma_start(out=ct_out[:].rearrange("(fc p) b -> p fc b", p=128), in_=ct_s[:])

            # ---- p_gen ----
            ps_pg = pss.tile([1, BL], F32, space="PSUM", tag="ps_small")
            pg_rhs = [ct_s[:, k, :] for k in range(8)] + \
                     [hT_s[:, k, :] for k in range(4)] + \
                     [cT_s[:, k, :] for k in range(4)] + [xs[:]]
            for k in range(17):
                nc.tensor.matmul(out=ps_pg[:], lhsT=wpg_t[:, k:k + 1], rhs=pg_rhs[k],
                                 start=(k == 0), stop=(k == 16))
            pg_s = fp.tile([1, BL], F32, tag="pg")
            nc.scalar.activation(out=pg_s[:], in_=ps_pg[:], func=AF.Tanh,
                                 bias=bpgh_t[:, 0:1], scale=0.5)
            nc.vector.tensor_scalar(out=pg_s[:], in0=pg_s[:], scalar1=0.5, scalar2=0.5,
                                    op0=ALU.mult, op1=ALU.add)
            nc.sync.dma_start(out=pg_out[:].rearrange("b x -> x b"), in_=pg_s[:])

            # ---- s2 = [h; c_t] @ W_o1.T + b_o1 ----
            s2_s = fp.tile([128, 4, BL], F32, tag="s2")
            for mc in range(4):
                ps_s2 = pss.tile([128, BL], F32, space="PSUM", tag="ps_small")
                for c in range(12):
                    rhs = hT_s[:, c, :] if c < 4 else ct_s[:, c - 4, :]
                    nc.tensor.matmul(out=ps_s2[:], lhsT=wo1_t[:, c, mc * 128:(mc + 1) * 128],
                                     rhs=rhs, start=(c == 0), stop=(c == 11))
                nc.vector.tensor_scalar_add(out=s2_s[:, mc, :], in0=ps_s2[:],
                                            scalar1=bo1_t[:, mc:mc + 1])
            nc.sync.dma_start(out=s2_out[:].rearrange("(c p) b -> p c b", p=128), in_=s2_s[:])

    _split_multi_waits(nc)
    return nc


def build_program_b():
    """Vocab projection + exp, tensor-parallel over 6250 vocab columns."""
    nc = bass.Bass()
    s2gT = nc.dram_tensor("s2gT", [4, 128, B], BF, kind="ExternalInput")   # s2_full.T chunks
    Wo2 = nc.dram_tensor("Wo2", [4, 128, VL], BF, kind="ExternalInput")    # W_o2.T chunks
    bo2 = nc.dram_tensor("bo2", [1, VL], BF, kind="ExternalInput")
    expv_out = nc.dram_tensor("expv_out", [B, VL], F32, kind="ExternalOutput")
    vsum_out = nc.dram_tensor("vsum_out", [B, 1], F32, kind="ExternalOutput")

    with tile.TileContext(nc) as tc:
        with (
            tc.tile_pool(name="wp", bufs=1) as wp,
            tc.tile_pool(name="wo2s", bufs=2) as wo2p,
            tc.tile_pool(name="psb", bufs=2, space="PSUM") as psb,
        ):
            s2g = wp.tile([128, 4, B], BF)
            nc.sync.dma_start(out=s2g[:], in_=s2gT[:].rearrange("c p b -> p c b"))
            bo2_t = wp.tile([1, VL], BF)
            nc.sync.dma_start(out=bo2_t[:], in_=bo2[:])
            ones1 = wp.tile([1, 128], BF)
            nc.vector.memset(ones1[:], 1.0)
            expv = wp.tile([128, VL], F32)
            vsum = wp.tile([128, len(NCH)], F32)
            GW = 2048  # W_o2 columns per prefetch group (4 PSUM chunks)
            grps = [(g, min(GW, VL - g)) for g in range(0, VL, GW)]
            for glo, gw in grps:
                wo2c = wo2p.tile([128, 4, GW], BF, tag="wo2g")
                nc.sync.dma_start(out=wo2c[:, :, :gw],
                                  in_=Wo2[:, :, glo:glo + gw].rearrange("c p n -> p c n"))
                for off in range(0, gw, 512):
                    lo = glo + off
                    w = min(512, gw - off)
                    i = lo // 512
                    ps_o = psb.tile([128, 512], F32, space="PSUM", tag="ps_o")
                    for kc in range(4):
                        nc.tensor.matmul(out=ps_o[:, :w], lhsT=s2g[:, kc, :],
                                         rhs=wo2c[:, kc, off:off + w],
                                         start=(kc == 0), stop=False)
                    nc.tensor.matmul(out=ps_o[:, :w], lhsT=ones1[:], rhs=bo2_t[:, lo:lo + w],
                                     start=False, stop=True)
                    nc.scalar.activation(out=expv[:, lo:lo + w], in_=ps_o[:, :w], func=AF.Exp,
                                         accum_out=vsum[:, i:i + 1])
            vsum_t = wp.tile([128, 1], F32)
            nc.vector.reduce_sum(out=vsum_t[:], in_=vsum[:], axis=mybir.AxisListType.X)
            nc.sync.dma_start(out=vsum_out[:], in_=vsum_t[:])
            nc.sync.dma_start(out=expv_out[:], in_=expv[:])

    _split_multi_waits(nc)
    return nc


def build_program_c():
    """final_cols = expv * (p_gen / S) — the cross-core-normalized scale."""
    nc = bass.Bass()
    expv_in = nc.dram_tensor("expv_in", [B, VL], F32, kind="ExternalInput")
    scale_in = nc.dram_tensor("scale_in", [B, 1], F32, kind="ExternalInput")
    final_cols = nc.dram_tensor("final_cols", [B, VL], F32, kind="ExternalOutput")
    with tile.TileContext(nc) as tc:
        with tc.tile_pool(name="p", bufs=1) as p:
            ev = p.tile([128, VL], F32)
            nc.sync.dma_start(out=ev[:], in_=expv_in[:])
            sc = p.tile([128, 1], F32)
            nc.sync.dma_start(out=sc[:], in_=scale_in[:])
            nc.vector.tensor_scalar_mul(out=ev[:], in0=ev[:], scalar1=sc[:, 0:1])
            nc.sync.dma_start(out=final_cols[:], in_=ev[:])
    _split_multi_waits(nc)
    return nc


# ---------------------------------------------------------------------------
_PROGRAMS: dict = {}


def _get_program(key, builder, *args):
    if key not in _PROGRAMS:
        _PROGRAMS[key] = builder(*args)
    return _PROGRAMS[key]


def _bf(x):
    return np.asarray(x, np.float32).astype(ml_dtypes.bfloat16)


def prep_in_maps(y_t_1, h0, c0, c_t_1, encoder_outputs, encoder_feature, mask_select,
                 enc_batch_extend_vocab, coverage, emb_table, W_c, W_dp, b_dp, v_w,
                 W_xc, b_xc, W_ih, W_hh, b_ih, b_hh, W_pg, b_pg, W_o1, b_o1, W_o2, b_o2):
    to32 = lambda a: np.asarray(a, np.float32)
    y_t_1 = np.asarray(y_t_1)
    h0, c0, c_t_1 = to32(h0), to32(c0), to32(c_t_1)
    encoder_outputs, encoder_feature = to32(encoder_outputs), to32(encoder_feature)
    mask_select, coverage, emb_table = to32(mask_select), to32(coverage), to32(emb_table)
    W_c, W_dp, b_dp, v_w = to32(W_c), to32(W_dp), to32(b_dp), to32(v_w)
    W_xc, b_xc, W_ih, W_hh = to32(W_xc), to32(b_xc), to32(W_ih), to32(W_hh)
    b_ih, b_hh, W_pg, b_pg = to32(b_ih), to32(b_hh), to32(W_pg), to32(b_pg)
    W_o1, b_o1, W_o2, b_o2 = to32(W_o1), to32(b_o1), to32(W_o2), to32(b_o2)

    y_emb = emb_table[y_t_1]                                   # [B, E]
    shared = dict(
        Wxc=np.ascontiguousarray(W_xc.T),
        bxc=b_xc.reshape(E, 1),
        Wih=np.ascontiguousarray(W_ih.T),
        Whh=np.ascontiguousarray(W_hh.T),
        bih2=np.ascontiguousarray(b_ih.reshape(16, 128).T),
        bhh2=np.ascontiguousarray(b_hh.reshape(16, 128).T),
        Wdp=np.ascontiguousarray(W_dp.T),
        bdp2=np.ascontiguousarray(b_dp.reshape(8, 128).T),
        vw2=_bf(v_w[0].reshape(8, 128).T),
        wcrow=_bf(W_c[:, 0].reshape(1, H2)),
        Wpg=np.ascontiguousarray(W_pg[0].reshape(17, 128).T),
        bpg=b_pg.reshape(1, 1),
        Wo1=np.ascontiguousarray(W_o1.T),
        bo12=np.ascontiguousarray(b_o1.reshape(4, 128).T),
    )
    in_maps = []
    for k in range(NCORES):
        bsl = slice(k * BL, (k + 1) * BL)
        m = dict(shared)
        m["xinT"] = np.ascontiguousarray(
            np.concatenate([c_t_1[bsl], y_emb[bsl]], axis=1).T)
        m["h0T"] = np.ascontiguousarray(h0[bsl].T)
        m["c0T"] = np.ascontiguousarray(c0[bsl].T)
        m["encfT"] = _bf(np.ascontiguousarray(encoder_feature[bsl].transpose(0, 2, 1)))
        m["enco"] = _bf(encoder_outputs[bsl])
        m["mask"] = np.ascontiguousarray(mask_select[bsl])
        m["cov"] = np.ascontiguousarray(coverage[bsl])
        m["covrow"] = _bf(coverage[bsl].reshape(1, BL * T))
        in_maps.append(m)

    Wo2T_bf = _bf(np.ascontiguousarray(W_o2.T)).reshape(4, 128, V)
    bo2_bf = _bf(b_o2.reshape(1, V))
    b_maps_wo2 = []
    for k in range(NCORES):
        vsl = slice(k * VL, (k + 1) * VL)
        b_maps_wo2.append(dict(Wo2=np.ascontiguousarray(Wo2T_bf[:, :, vsl]),
                               bo2=np.ascontiguousarray(bo2_bf[:, vsl])))
    use_cov = bool(np.any(coverage != 0.0))
    return in_maps, b_maps_wo2, use_cov, enc_batch_extend_vocab


def _run_spmd(nc, in_maps):
    from concourse.bass_utils import run_bass_kernel_spmd
    return run_bass_kernel_spmd(nc, in_maps, list(range(NCORES))).results


RUN_WALL = {}


def kernel(**inputs):
    import time as _time
    in_maps, b_maps_wo2, use_cov, ebv = prep_in_maps(**inputs)
    nc_a = _get_program(("a", use_cov), build_program_a, use_cov)
    nc_b = _get_program("b", build_program_b)
    nc_c = _get_program("c", build_program_c)

    t0 = _time.time()
    res_a = _run_spmd(nc_a, in_maps)
    RUN_WALL["a"] = _time.time() - t0

    s2_full = np.concatenate([np.asarray(res_a[k]["s2_out"]).T for k in range(NCORES)], 0)
    pg_full = np.concatenate([np.asarray(res_a[k]["pg_out"]) for k in range(NCORES)], 0)
    s2gT = _bf(np.stack([s2_full[:, kc * 128:(kc + 1) * 128].T for kc in range(4)], 0))
    b_in = [dict(s2gT=s2gT, **b_maps_wo2[k]) for k in range(NCORES)]

    t0 = _time.time()
    res_b = _run_spmd(nc_b, b_in)
    RUN_WALL["b"] = _time.time() - t0

    S = np.sum([np.asarray(res_b[k]["vsum_out"]) for k in range(NCORES)], axis=0)  # [B,1]
    scale = (pg_full / S).astype(np.float32)
    c_in = [dict(expv_in=np.asarray(res_b[k]["expv_out"]), scale_in=scale)
            for k in range(NCORES)]

    t0 = _time.time()
    res_c = _run_spmd(nc_c, c_in)
    RUN_WALL["c"] = _time.time() - t0

    final = np.concatenate([np.asarray(res_c[k]["final_cols"]) for k in range(NCORES)], 1)
    h = np.concatenate([np.asarray(res_a[k]["h_out"]).T for k in range(NCORES)], 0)
    c = np.concatenate([np.asarray(res_a[k]["c_out"]).T for k in range(NCORES)], 0)
    c_t = np.concatenate([np.asarray(res_a[k]["ct_out"]).T for k in range(NCORES)], 0)
    attn = np.concatenate([np.asarray(res_a[k]["attn_out"]) for k in range(NCORES)], 0)
    covn = np.concatenate([np.asarray(res_a[k]["covn_out"]) for k in range(NCORES)], 0)

    # pointer scatter-add applied during unshard (host), from device outputs
    rows = np.arange(B)[:, None]
    np.add.at(final, (rows, np.asarray(ebv)), (1.0 - pg_full) * attn)
    return final, h, c, c_t, attn, pg_full, covn


# revision 13
# speedup vs baseline: 1.0957x; 1.0957x over previous
"""Trainium2 Bass kernel for nn_Decoder_65498251264356.

Pointer-generator decoder step: embedding + LSTM cell + Bahdanau attention
(with coverage) + pointer-gate + vocab softmax + pointer scatter-mix.

Sharding: data-parallel over batch B=128 across 8 cores (16 rows each) for
the LSTM/attention front (launch A); the vocab projection is
tensor-parallel over V=50000 (6250 columns per core, launch B) using the
s2/p_gen rows gathered on the host between launches; the vocab softmax
normalizer is an 8-way partial-sum exchange through the host, applied
on-device in launch C. The final pointer scatter-add is applied on the
host during unsharding, using device-computed attn and p_gen.
(Device collectives compile but fail to load under this axon terminal, so
cross-core exchanges ride the host launch boundaries instead.)
"""
import sys

sys.path.insert(0, "/opt/trn_rl_repo")

import numpy as np
import ml_dtypes

import concourse.bass as bass
import concourse.mybir as mybir
import concourse.tile as tile
from concourse.masks import make_identity
from concourse.vector_clock import ScopedClock

dt = mybir.dt
AF = mybir.ActivationFunctionType
ALU = mybir.AluOpType

B, T, H, E, V = 128, 400, 512, 128, 50000
NCORES = 8
BL = B // NCORES        # 16 batch rows per core
VL = V // NCORES        # 6250 vocab cols per core
H2 = 2 * H              # 1024
EPS = 1e-12
BF = dt.bfloat16
F32 = dt.float32

NCH = [(i * 512, min(512, VL - i * 512)) for i in range((VL + 511) // 512)]


# ---------------------------------------------------------------------------
# walrus in this container rejects >1 sem wait per instruction; split the
# Tile tail-drain's aggregated waits onto single-wait NOPs.
def _patched_drain_and_barrier(self, tick_clock, wait_clock):
    nc = self.nc
    carrier = nc.sync.nop(nofuse=True)
    wait_clock.add_sem_waits(carrier.ins, ScopedClock({None: tick_clock.global_clock}))
    si = carrier.ins.sync_info
    waits = list(si.on_wait or []) if si else []
    if len(waits) > 1:
        carrier.ins.sync_info = mybir.SyncInfo(
            on_wait=waits[:1], on_update=list(si.on_update or [])
        )
        for w in waits[1:]:
            n = nc.sync.nop(nofuse=True)
            n.ins.sync_info = mybir.SyncInfo(on_wait=[w], on_update=[])
    nc.sync.drain()
    nc.all_engine_barrier()
    assert self.sems is not None
    popped = nc._tile_sem_poison_stack.pop()
    assert popped is self._sem_poison
    nc.clear_and_free_semaphores(list(self.sems.allocated().values()))
    nc.all_engine_barrier()


tile.TileContext._drain_and_barrier = _patched_drain_and_barrier

SPLIT_WAITS = True
_wsplit_ctr = [0]


def _split_multi_waits(nc):
    if not SPLIT_WAITS:
        return
    """Same walrus limit, applied globally: extra waits move onto
    single-wait NOPs inserted just before the instruction, same engine."""
    for f in nc.m.functions:
        for bb in f.blocks:
            il = bb.instructions
            i = 0
            while i < len(il):
                inst = il[i]
                si = inst.sync_info
                waits = list(si.on_wait) if si and si.on_wait else []
                if len(waits) > 1:
                    for w in waits[:-1]:
                        _wsplit_ctr[0] += 1
                        nop = mybir.InstNoOp(
                            name=f"I-wsplit-{_wsplit_ctr[0]}",
                            engine=inst.engine,
                            sync_info=mybir.SyncInfo(on_wait=[w], on_update=[]),
                        )
                        il.insert(i, nop)
                        i += 1
                    inst.sync_info = mybir.SyncInfo(
                        on_wait=[waits[-1]], on_update=list(si.on_update or [])
                    )
                i += 1
# ---------------------------------------------------------------------------


def build_program_a(use_cov: bool):
    """LSTM + attention + p_gen + s2, data-parallel over 16 batch rows."""
    nc = bass.Bass()

    xinT = nc.dram_tensor("xinT", [9 * E, BL], F32, kind="ExternalInput")   # [c_t_1; emb].T
    h0T = nc.dram_tensor("h0T", [H, BL], F32, kind="ExternalInput")
    c0T = nc.dram_tensor("c0T", [H, BL], F32, kind="ExternalInput")
    encfT = nc.dram_tensor("encfT", [BL, H2, T], BF, kind="ExternalInput")  # feature-major
    enco = nc.dram_tensor("enco", [BL, T, H2], BF, kind="ExternalInput")    # natural
    mask = nc.dram_tensor("mask", [BL, T], F32, kind="ExternalInput")
    cov = nc.dram_tensor("cov", [BL, T], F32, kind="ExternalInput")
    Wxc = nc.dram_tensor("Wxc", [9 * E, E], F32, kind="ExternalInput")      # W_xc.T
    bxc = nc.dram_tensor("bxc", [E, 1], F32, kind="ExternalInput")
    Wih = nc.dram_tensor("Wih", [E, 4 * H], F32, kind="ExternalInput")      # W_ih.T
    Whh = nc.dram_tensor("Whh", [H, 4 * H], F32, kind="ExternalInput")      # W_hh.T
    bih2 = nc.dram_tensor("bih2", [128, 16], F32, kind="ExternalInput")
    bhh2 = nc.dram_tensor("bhh2", [128, 16], F32, kind="ExternalInput")
    Wdp = nc.dram_tensor("Wdp", [H2, H2], F32, kind="ExternalInput")        # W_dp.T
    bdp2 = nc.dram_tensor("bdp2", [128, 8], F32, kind="ExternalInput")
    vw2 = nc.dram_tensor("vw2", [128, 8], BF, kind="ExternalInput")         # v chunks
    wcrow = nc.dram_tensor("wcrow", [1, H2], BF, kind="ExternalInput")      # W_c row
    covrow = nc.dram_tensor("covrow", [1, BL * T], BF, kind="ExternalInput")
    Wpg = nc.dram_tensor("Wpg", [128, 17], F32, kind="ExternalInput")       # W_pg.T chunks
    bpg = nc.dram_tensor("bpg", [1, 1], F32, kind="ExternalInput")
    Wo1 = nc.dram_tensor("Wo1", [3 * H, H], F32, kind="ExternalInput")      # W_o1.T
    bo12 = nc.dram_tensor("bo12", [128, 4], F32, kind="ExternalInput")

    h_out = nc.dram_tensor("h_out", [H, BL], F32, kind="ExternalOutput")
    c_out = nc.dram_tensor("c_out", [H, BL], F32, kind="ExternalOutput")
    ct_out = nc.dram_tensor("ct_out", [H2, BL], F32, kind="ExternalOutput")
    attn_out = nc.dram_tensor("attn_out", [BL, T], F32, kind="ExternalOutput")
    pg_out = nc.dram_tensor("pg_out", [BL, 1], F32, kind="ExternalOutput")
    covn_out = nc.dram_tensor("covn_out", [BL, T], F32, kind="ExternalOutput")
    s2_out = nc.dram_tensor("s2_out", [H, BL], F32, kind="ExternalOutput")

    with tile.TileContext(nc) as tc:
        with (
            tc.tile_pool(name="wp", bufs=1) as wp,
            tc.tile_pool(name="fp", bufs=1) as fp,
            tc.tile_pool(name="ap", bufs=3) as ap,
            tc.tile_pool(name="pss", bufs=2, space="PSUM") as pss,
            tc.tile_pool(name="pssc", bufs=2, space="PSUM") as pssc,
            tc.tile_pool(name="psct", bufs=1, space="PSUM") as psct,
        ):
            # ---- persistent small tiles ----
            vw_t = wp.tile([128, 8], BF)
            nc.sync.dma_start(out=vw_t[:], in_=vw2[:])
            bxc_t = wp.tile([128, 1], F32)
            nc.sync.dma_start(out=bxc_t[:], in_=bxc[:])
            bih_t = wp.tile([128, 16], F32)
            nc.sync.dma_start(out=bih_t[:], in_=bih2[:])
            bhh_t = wp.tile([128, 16], F32)
            nc.sync.dma_start(out=bhh_t[:], in_=bhh2[:])
            bdp_t = wp.tile([128, 8], F32)
            nc.sync.dma_start(out=bdp_t[:], in_=bdp2[:])
            bo1_t = wp.tile([128, 4], F32)
            nc.sync.dma_start(out=bo1_t[:], in_=bo12[:])
            bpg_t = wp.tile([1, 1], F32)
            nc.sync.dma_start(out=bpg_t[:], in_=bpg[:])
            wpg_t = wp.tile([128, 17], F32)
            nc.sync.dma_start(out=wpg_t[:], in_=Wpg[:])
            mask_t = wp.tile([BL, T], F32)
            nc.sync.dma_start(out=mask_t[:], in_=mask[:])
            cov_t = wp.tile([BL, T], F32)
            nc.sync.dma_start(out=cov_t[:], in_=cov[:])
            if use_cov:
                wcrow_t = wp.tile([1, H2], BF)
                nc.sync.dma_start(out=wcrow_t[:], in_=wcrow[:])
                covrow_t = wp.tile([1, BL * T], BF)
                nc.sync.dma_start(out=covrow_t[:], in_=covrow[:])
            ident = wp.tile([128, 128], F32)
            make_identity(nc, ident[:])

            # combined LSTM gate bias, plus halved for sigmoid-via-tanh:
            # sigmoid(x) = 0.5*tanh(0.5*x) + 0.5
            bg_t = wp.tile([128, 16], F32)
            nc.vector.tensor_tensor(out=bg_t[:], in0=bih_t[:], in1=bhh_t[:], op=ALU.add)
            bgh_t = wp.tile([128, 16], F32)
            nc.vector.tensor_scalar_mul(out=bgh_t[:], in0=bg_t[:], scalar1=0.5)
            bpgh_t = wp.tile([1, 1], F32)
            nc.vector.tensor_scalar_mul(out=bpgh_t[:], in0=bpg_t[:], scalar1=0.5)

            # ---- big front weights (recycled slots) ----
            wxc_t = fp.tile([128, 9, E], F32, tag="wxc")
            nc.gpsimd.dma_start(out=wxc_t[:], in_=Wxc[:].rearrange("(c p) m -> p c m", p=128))
            wih_t = fp.tile([128, 4 * H], F32, tag="wih")
            nc.gpsimd.dma_start(out=wih_t[:], in_=Wih[:])
            whh_t = fp.tile([128, 4, 4 * H], F32, tag="whh")
            nc.gpsimd.dma_start(out=whh_t[:], in_=Whh[:].rearrange("(c p) m -> p c m", p=128))
            wdp_t = fp.tile([128, 8, H2], F32, tag="wdp")
            nc.gpsimd.dma_start(out=wdp_t[:], in_=Wdp[:].rearrange("(c p) m -> p c m", p=128))
            wo1_t = fp.tile([128, 12, H], F32, tag="wo1")
            nc.gpsimd.dma_start(out=wo1_t[:], in_=Wo1[:].rearrange("(c p) m -> p c m", p=128))

            # ---- x = [c_t_1; emb] @ W_xc.T + b ----
            xin_t = fp.tile([128, 9, BL], F32, tag="xin")
            nc.sync.dma_start(out=xin_t[:], in_=xinT[:].rearrange("(c p) b -> p c b", p=128))
            h0_t = fp.tile([128, 4, BL], F32, tag="h0")
            nc.sync.dma_start(out=h0_t[:], in_=h0T[:].rearrange("(c p) b -> p c b", p=128))
            c0_t = fp.tile([128, 4, BL], F32, tag="c0")
            nc.sync.dma_start(out=c0_t[:], in_=c0T[:].rearrange("(c p) b -> p c b", p=128))

            ps_x = pss.tile([128, BL], F32, space="PSUM", tag="ps_small")
            for c in range(9):
                nc.tensor.matmul(out=ps_x[:], lhsT=wxc_t[:, c, :], rhs=xin_t[:, c, :],
                                 start=(c == 0), stop=(c == 8))
            xs = fp.tile([128, BL], F32, tag="xs")
            nc.vector.tensor_scalar_add(out=xs[:], in0=ps_x[:], scalar1=bxc_t[:, 0:1])

            # ---- LSTM gates (order i,f,g,o) ----
            i_s = fp.tile([128, 4, BL], F32, tag="i_s")
            f_s = fp.tile([128, 4, BL], F32, tag="f_s")
            g_s = fp.tile([128, 4, BL], F32, tag="g_s")
            o_s = fp.tile([128, 4, BL], F32, tag="o_s")
            gate_dst = [i_s, f_s, g_s, o_s]
            for mc in range(16):
                ps_g = pss.tile([128, BL], F32, space="PSUM", tag="ps_small")
                nc.tensor.matmul(out=ps_g[:], lhsT=wih_t[:, mc * 128:(mc + 1) * 128],
                                 rhs=xs[:], start=True, stop=False)
                for c in range(4):
                    nc.tensor.matmul(out=ps_g[:], lhsT=whh_t[:, c, mc * 128:(mc + 1) * 128],
                                     rhs=h0_t[:, c, :], start=False, stop=(c == 3))
                dst = gate_dst[mc // 4][:, mc % 4, :]
                if mc // 4 == 2:  # g -> tanh(x + b)
                    nc.scalar.activation(out=dst, in_=ps_g[:], func=AF.Tanh,
                                         bias=bg_t[:, mc:mc + 1], scale=1.0)
                else:  # i,f,o -> sigmoid via tanh
                    nc.scalar.activation(out=dst, in_=ps_g[:], func=AF.Tanh,
                                         bias=bgh_t[:, mc:mc + 1], scale=0.5)
            for gidx in (0, 1, 3):
                g = gate_dst[gidx]
                nc.vector.tensor_scalar(out=g[:], in0=g[:], scalar1=0.5, scalar2=0.5,
                                        op0=ALU.mult, op1=ALU.add)

            # ---- c, h ----
            cT_s = fp.tile([128, 4, BL], F32, tag="cT")
            hT_s = fp.tile([128, 4, BL], F32, tag="hT")
            tnc = fp.tile([128, 4, BL], F32, tag="tnc")
            for c in range(4):
                t1 = fp.tile([128, BL], F32, tag="lstm_t1")
                nc.vector.tensor_tensor(out=t1[:], in0=f_s[:, c, :], in1=c0_t[:, c, :], op=ALU.mult)
                t2 = fp.tile([128, BL], F32, tag="lstm_t2")
                nc.vector.tensor_tensor(out=t2[:], in0=i_s[:, c, :], in1=g_s[:, c, :], op=ALU.mult)
                nc.vector.tensor_tensor(out=cT_s[:, c, :], in0=t1[:], in1=t2[:], op=ALU.add)
                nc.scalar.activation(out=tnc[:, c, :], in_=cT_s[:, c, :], func=AF.Tanh)
                nc.vector.tensor_tensor(out=hT_s[:, c, :], in0=o_s[:, c, :], in1=tnc[:, c, :], op=ALU.mult)

            nc.sync.dma_start(out=h_out[:].rearrange("(c p) b -> p c b", p=128), in_=hT_s[:])
            nc.sync.dma_start(out=c_out[:].rearrange("(c p) b -> p c b", p=128), in_=cT_s[:])

            # ---- dec_fea = s_t_hat @ W_dp.T + b_dp ----
            dec_t = fp.tile([128, 8, BL], F32, tag="dec")
            for mc in range(8):
                ps_d = pss.tile([128, BL], F32, space="PSUM", tag="ps_small")
                for c in range(8):
                    rhs = hT_s[:, c, :] if c < 4 else cT_s[:, c - 4, :]
                    nc.tensor.matmul(out=ps_d[:], lhsT=wdp_t[:, c, mc * 128:(mc + 1) * 128],
                                     rhs=rhs, start=(c == 0), stop=(c == 7))
                nc.vector.tensor_scalar_add(out=dec_t[:, mc, :], in0=ps_d[:],
                                            scalar1=bdp_t[:, mc:mc + 1])

            # ---- attention pass 1: scores -> softmax ----
            # (engines may only address partition strips at 0/32/64/96, so
            # per-b exp results land on partition 0 as column slices and are
            # redistributed to [BL, T] with one SBUF->SBUF DMA)
            expsc_row = fp.tile([1, BL * T], F32, tag="expsc_row")
            ssum_row = fp.tile([1, BL], F32, tag="ssum_row")
            for b in range(BL):
                eF = ap.tile([128, 8, T], BF, tag="eF")
                nc.sync.dma_start(out=eF[:], in_=encfT[b].rearrange("(c p) t -> p c t", p=128))
                eE = ap.tile([128, 8, T], BF, tag="eE")
                if use_cov:
                    for c in range(8):
                        ps_cov = pssc.tile([128, T], F32, space="PSUM", tag="ps_cov")
                        nc.tensor.matmul(out=ps_cov[:], lhsT=wcrow_t[:, c * 128:(c + 1) * 128],
                                         rhs=covrow_t[:, b * T:(b + 1) * T], start=True, stop=True)
                        t3 = ap.tile([128, T], F32, tag="covtmp")
                        nc.vector.tensor_scalar_add(out=t3[:], in0=eF[:, c, :],
                                                    scalar1=dec_t[:, c, b:b + 1])
                        nc.vector.tensor_tensor(out=eE[:, c, :], in0=t3[:], in1=ps_cov[:], op=ALU.add)
                else:
                    for c in range(8):
                        nc.vector.tensor_scalar_add(out=eE[:, c, :], in0=eF[:, c, :],
                                                    scalar1=dec_t[:, c, b:b + 1])
                nc.scalar.activation(out=eE[:], in_=eE[:], func=AF.Tanh)
                ps_sc = pssc.tile([1, T], F32, space="PSUM", tag="ps_sc")
                for c in range(8):
                    nc.tensor.matmul(out=ps_sc[:], lhsT=vw_t[:, c:c + 1], rhs=eE[:, c, :],
                                     start=(c == 0), stop=(c == 7))
                nc.scalar.activation(out=expsc_row[:, b * T:(b + 1) * T], in_=ps_sc[:],
                                     func=AF.Exp, accum_out=ssum_row[:, b:b + 1])

            expsc = fp.tile([BL, T], F32, tag="expsc")
            nc.sync.dma_start(out=expsc[:], in_=expsc_row[:].rearrange("x (b t) -> x b t", b=BL))
            ssum = fp.tile([BL, 1], F32, tag="ssum")
            nc.sync.dma_start(out=ssum[:], in_=ssum_row[:].rearrange("x (b o) -> x b o", b=BL))

            # softmax tail; equals attn_/(sum(attn_)+eps), attn_ = softmax*mask
            m1 = fp.tile([BL, T], F32, tag="m1")
            nc.vector.tensor_tensor(out=m1[:], in0=expsc[:], in1=mask_t[:], op=ALU.mult)
            s1 = fp.tile([BL, 1], F32, tag="s1")
            nc.vector.reduce_sum(out=s1[:], in_=m1[:], axis=mybir.AxisListType.X)
            den = fp.tile([BL, 1], F32, tag="den")
            nc.vector.tensor_scalar_mul(out=den[:], in0=ssum[:], scalar1=float(EPS))
            nc.vector.tensor_tensor(out=den[:], in0=den[:], in1=s1[:], op=ALU.add)
            rden = fp.tile([BL, 1], F32, tag="rden")
            nc.vector.reciprocal(out=rden[:], in_=den[:])
            attn_t = fp.tile([BL, T], F32, tag="attn")
            nc.vector.tensor_scalar_mul(out=attn_t[:], in0=m1[:], scalar1=rden[:, 0:1])
            nc.sync.dma_start(out=attn_out[:], in_=attn_t[:])
            covn_t = fp.tile([BL, T], F32, tag="covn")
            nc.vector.tensor_tensor(out=covn_t[:], in0=cov_t[:], in1=attn_t[:], op=ALU.add)
            nc.sync.dma_start(out=covn_out[:], in_=covn_t[:])

            # attn.T chunks for c_t matmuls (PE transpose), bf16.
            # T=400 wraps as 4 chunks of 100 partitions so enco[b] loads in
            # ONE DMA below (no ragged 16-row tail transfer).
            attnT_s = fp.tile([100, 4, BL], BF, tag="attnT")
            for q in range(4):
                lo = q * 100
                ps_tr = pss.tile([128, BL], F32, space="PSUM", tag="ps_small")
                nc.tensor.transpose(out=ps_tr[:100, :], in_=attn_t[:, lo:lo + 100],
                                    identity=ident[:BL, :BL])
                nc.vector.tensor_copy(out=attnT_s[:, q, :], in_=ps_tr[:100, :])

            # ---- attention pass 2: c_t ----
            ps_ct = psct.tile([128, 8 * BL], F32, space="PSUM")
            for b in range(BL):
                oT = ap.tile([100, 4, H2], BF, tag="oT")
                nc.gpsimd.dma_start(out=oT[:], in_=enco[b].rearrange("(q p) f -> p q f", p=100))
                for fc in range(8):
                    col = fc * BL + b
                    for q in range(4):
                        nc.tensor.matmul(out=ps_ct[:, col:col + 1],
                                         lhsT=oT[:, q, fc * 128:(fc + 1) * 128],
                                         rhs=attnT_s[:, q, b:b + 1],
                                         start=(q == 0), stop=(q == 3))
            ct_s = fp.tile([128, 8, BL], F32, tag="ct")
            nc.vector.tensor_copy(out=ct_s[:], in_=ps_ct[:].rearrange("p (fc b) -> p fc b", fc=8))
            nc.sync.dma_start(out=ct_out[:].rearrange("(fc p) b -> p fc b", p=128), in_=ct_s[:])

            # ---- p_gen ----
            ps_pg = pss.tile([1, BL], F32, space="PSUM", tag="ps_small")
            pg_rhs = [ct_s[:, k, :] for k in range(8)] + \
                     [hT_s[:, k, :] for k in range(4)] + \
                     [cT_s[:, k, :] for k in range(4)] + [xs[:]]
            for k in range(17):
                nc.tensor.matmul(out=ps_pg[:], lhsT=wpg_t[:, k:k + 1], rhs=pg_rhs[k],
                                 start=(k == 0), stop=(k == 16))
            pg_s = fp.tile([1, BL], F32, tag="pg")
            nc.scalar.activation(out=pg_s[:], in_=ps_pg[:], func=AF.Tanh,
                                 bias=bpgh_t[:, 0:1], scale=0.5)
            nc.vector.tensor_scalar(out=pg_s[:], in0=pg_s[:], scalar1=0.5, scalar2=0.5,
                                    op0=ALU.mult, op1=ALU.add)
            nc.sync.dma_start(out=pg_out[:].rearrange("b x -> x b"), in_=pg_s[:])

            # ---- s2 = [h; c_t] @ W_o1.T + b_o1 ----
            s2_s = fp.tile([128, 4, BL], F32, tag="s2")
            for mc in range(4):
                ps_s2 = pss.tile([128, BL], F32, space="PSUM", tag="ps_small")
                for c in range(12):
                    rhs = hT_s[:, c, :] if c < 4 else ct_s[:, c - 4, :]
                    nc.tensor.matmul(out=ps_s2[:], lhsT=wo1_t[:, c, mc * 128:(mc + 1) * 128],
                                     rhs=rhs, start=(c == 0), stop=(c == 11))
                nc.vector.tensor_scalar_add(out=s2_s[:, mc, :], in0=ps_s2[:],
                                            scalar1=bo1_t[:, mc:mc + 1])
            nc.sync.dma_start(out=s2_out[:].rearrange("(c p) b -> p c b", p=128), in_=s2_s[:])

    _split_multi_waits(nc)
    return nc


def build_program_b():
    """Vocab projection + exp, tensor-parallel over 6250 vocab columns."""
    nc = bass.Bass()
    s2gT = nc.dram_tensor("s2gT", [4, 128, B], BF, kind="ExternalInput")   # s2_full.T chunks
    Wo2 = nc.dram_tensor("Wo2", [4, 128, VL], BF, kind="ExternalInput")    # W_o2.T chunks
    bo2 = nc.dram_tensor("bo2", [1, VL], BF, kind="ExternalInput")
    expv_out = nc.dram_tensor("expv_out", [B, VL], F32, kind="ExternalOutput")
    vsum_out = nc.dram_tensor("vsum_out", [B, 1], F32, kind="ExternalOutput")

    with tile.TileContext(nc) as tc:
        with (
            tc.tile_pool(name="wp", bufs=1) as wp,
            tc.tile_pool(name="wo2s", bufs=2) as wo2p,
            tc.tile_pool(name="psb", bufs=2, space="PSUM") as psb,
        ):
            s2g = wp.tile([128, 4, B], BF)
            nc.sync.dma_start(out=s2g[:], in_=s2gT[:].rearrange("c p b -> p c b"))
            bo2_t = wp.tile([1, VL], BF)
            nc.sync.dma_start(out=bo2_t[:], in_=bo2[:])
            ones1 = wp.tile([1, 128], BF)
            nc.vector.memset(ones1[:], 1.0)
            expv = wp.tile([128, VL], F32)
            vsum = wp.tile([128, len(NCH)], F32)
            GW = 2048  # W_o2 columns per prefetch group (4 PSUM chunks)
            grps = [(g, min(GW, VL - g)) for g in range(0, VL, GW)]
            for gi, (glo, gw) in enumerate(grps):
                wo2c = wo2p.tile([128, 4, GW], BF, tag="wo2g")
                eng = nc.sync if gi % 2 == 0 else nc.gpsimd
                eng.dma_start(out=wo2c[:, :, :gw],
                              in_=Wo2[:, :, glo:glo + gw].rearrange("c p n -> p c n"))
                for off in range(0, gw, 512):
                    lo = glo + off
                    w = min(512, gw - off)
                    i = lo // 512
                    ps_o = psb.tile([128, 512], F32, space="PSUM", tag="ps_o")
                    for kc in range(4):
                        nc.tensor.matmul(out=ps_o[:, :w], lhsT=s2g[:, kc, :],
                                         rhs=wo2c[:, kc, off:off + w],
                                         start=(kc == 0), stop=False)
                    nc.tensor.matmul(out=ps_o[:, :w], lhsT=ones1[:], rhs=bo2_t[:, lo:lo + w],
                                     start=False, stop=True)
                    nc.scalar.activation(out=expv[:, lo:lo + w], in_=ps_o[:, :w], func=AF.Exp,
                                         accum_out=vsum[:, i:i + 1])
            vsum_t = wp.tile([128, 1], F32)
            nc.vector.reduce_sum(out=vsum_t[:], in_=vsum[:], axis=mybir.AxisListType.X)
            nc.sync.dma_start(out=vsum_out[:], in_=vsum_t[:])
            nc.sync.dma_start(out=expv_out[:], in_=expv[:])

    _split_multi_waits(nc)
    return nc


def build_program_c():
    """final_cols = expv * (p_gen / S) — the cross-core-normalized scale."""
    nc = bass.Bass()
    expv_in = nc.dram_tensor("expv_in", [B, VL], F32, kind="ExternalInput")
    scale_in = nc.dram_tensor("scale_in", [B, 1], F32, kind="ExternalInput")
    final_cols = nc.dram_tensor("final_cols", [B, VL], F32, kind="ExternalOutput")
    with tile.TileContext(nc) as tc:
        with tc.tile_pool(name="p", bufs=1) as p:
            ev = p.tile([128, VL], F32)
            nc.gpsimd.dma_start(out=ev[:], in_=expv_in[:])
            sc = p.tile([128, 1], F32)
            nc.sync.dma_start(out=sc[:], in_=scale_in[:])
            nc.vector.tensor_scalar_mul(out=ev[:], in0=ev[:], scalar1=sc[:, 0:1])
            nc.sync.dma_start(out=final_cols[:], in_=ev[:])
    _split_multi_waits(nc)
    return nc


# ---------------------------------------------------------------------------
_PROGRAMS: dict = {}


def _get_program(key, builder, *args):
    if key not in _PROGRAMS:
        _PROGRAMS[key] = builder(*args)
    return _PROGRAMS[key]


def _bf(x):
    return np.asarray(x, np.float32).astype(ml_dtypes.bfloat16)


def prep_in_maps(y_t_1, h0, c0, c_t_1, encoder_outputs, encoder_feature, mask_select,
                 enc_batch_extend_vocab, coverage, emb_table, W_c, W_dp, b_dp, v_w,
                 W_xc, b_xc, W_ih, W_hh, b_ih, b_hh, W_pg, b_pg, W_o1, b_o1, W_o2, b_o2):
    to32 = lambda a: np.asarray(a, np.float32)
    y_t_1 = np.asarray(y_t_1)
    h0, c0, c_t_1 = to32(h0), to32(c0), to32(c_t_1)
    encoder_outputs, encoder_feature = to32(encoder_outputs), to32(encoder_feature)
    mask_select, coverage, emb_table = to32(mask_select), to32(coverage), to32(emb_table)
    W_c, W_dp, b_dp, v_w = to32(W_c), to32(W_dp), to32(b_dp), to32(v_w)
    W_xc, b_xc, W_ih, W_hh = to32(W_xc), to32(b_xc), to32(W_ih), to32(W_hh)
    b_ih, b_hh, W_pg, b_pg = to32(b_ih), to32(b_hh), to32(W_pg), to32(b_pg)
    W_o1, b_o1, W_o2, b_o2 = to32(W_o1), to32(b_o1), to32(W_o2), to32(b_o2)

    y_emb = emb_table[y_t_1]                                   # [B, E]
    shared = dict(
        Wxc=np.ascontiguousarray(W_xc.T),
        bxc=b_xc.reshape(E, 1),
        Wih=np.ascontiguousarray(W_ih.T),
        Whh=np.ascontiguousarray(W_hh.T),
        bih2=np.ascontiguousarray(b_ih.reshape(16, 128).T),
        bhh2=np.ascontiguousarray(b_hh.reshape(16, 128).T),
        Wdp=np.ascontiguousarray(W_dp.T),
        bdp2=np.ascontiguousarray(b_dp.reshape(8, 128).T),
        vw2=_bf(v_w[0].reshape(8, 128).T),
        wcrow=_bf(W_c[:, 0].reshape(1, H2)),
        Wpg=np.ascontiguousarray(W_pg[0].reshape(17, 128).T),
        bpg=b_pg.reshape(1, 1),
        Wo1=np.ascontiguousarray(W_o1.T),
        bo12=np.ascontiguousarray(b_o1.reshape(4, 128).T),
    )
    in_maps = []
    for k in range(NCORES):
        bsl = slice(k * BL, (k + 1) * BL)
        m = dict(shared)
        m["xinT"] = np.ascontiguousarray(
            np.concatenate([c_t_1[bsl], y_emb[bsl]], axis=1).T)
        m["h0T"] = np.ascontiguousarray(h0[bsl].T)
        m["c0T"] = np.ascontiguousarray(c0[bsl].T)
        m["encfT"] = _bf(np.ascontiguousarray(encoder_feature[bsl].transpose(0, 2, 1)))
        m["enco"] = _bf(encoder_outputs[bsl])
        m["mask"] = np.ascontiguousarray(mask_select[bsl])
        m["cov"] = np.ascontiguousarray(coverage[bsl])
        m["covrow"] = _bf(coverage[bsl].reshape(1, BL * T))
        in_maps.append(m)

    Wo2T_bf = _bf(np.ascontiguousarray(W_o2.T)).reshape(4, 128, V)
    bo2_bf = _bf(b_o2.reshape(1, V))
    b_maps_wo2 = []
    for k in range(NCORES):
        vsl = slice(k * VL, (k + 1) * VL)
        b_maps_wo2.append(dict(Wo2=np.ascontiguousarray(Wo2T_bf[:, :, vsl]),
                               bo2=np.ascontiguousarray(bo2_bf[:, vsl])))
    use_cov = bool(np.any(coverage != 0.0))
    return in_maps, b_maps_wo2, use_cov, enc_batch_extend_vocab


def _run_spmd(nc, in_maps):
    from concourse.bass_utils import run_bass_kernel_spmd
    return run_bass_kernel_spmd(nc, in_maps, list(range(NCORES))).results


RUN_WALL = {}


def kernel(**inputs):
    import time as _time
    in_maps, b_maps_wo2, use_cov, ebv = prep_in_maps(**inputs)
    nc_a = _get_program(("a", use_cov), build_program_a, use_cov)
    nc_b = _get_program("b", build_program_b)
    nc_c = _get_program("c", build_program_c)

    t0 = _time.time()
    res_a = _run_spmd(nc_a, in_maps)
    RUN_WALL["a"] = _time.time() - t0

    s2_full = np.concatenate([np.asarray(res_a[k]["s2_out"]).T for k in range(NCORES)], 0)
    pg_full = np.concatenate([np.asarray(res_a[k]["pg_out"]) for k in range(NCORES)], 0)
    s2gT = _bf(np.stack([s2_full[:, kc * 128:(kc + 1) * 128].T for kc in range(4)], 0))
    b_in = [dict(s2gT=s2gT, **b_maps_wo2[k]) for k in range(NCORES)]

    t0 = _time.time()
    res_b = _run_spmd(nc_b, b_in)
    RUN_WALL["b"] = _time.time() - t0

    S = np.sum([np.asarray(res_b[k]["vsum_out"]) for k in range(NCORES)], axis=0)  # [B,1]
    scale = (pg_full / S).astype(np.float32)
    c_in = [dict(expv_in=np.asarray(res_b[k]["expv_out"]), scale_in=scale)
            for k in range(NCORES)]

    t0 = _time.time()
    res_c = _run_spmd(nc_c, c_in)
    RUN_WALL["c"] = _time.time() - t0

    final = np.concatenate([np.asarray(res_c[k]["final_cols"]) for k in range(NCORES)], 1)
    h = np.concatenate([np.asarray(res_a[k]["h_out"]).T for k in range(NCORES)], 0)
    c = np.concatenate([np.asarray(res_a[k]["c_out"]).T for k in range(NCORES)], 0)
    c_t = np.concatenate([np.asarray(res_a[k]["ct_out"]).T for k in range(NCORES)], 0)
    attn = np.concatenate([np.asarray(res_a[k]["attn_out"]) for k in range(NCORES)], 0)
    covn = np.concatenate([np.asarray(res_a[k]["covn_out"]) for k in range(NCORES)], 0)

    # pointer scatter-add applied during unshard (host), from device outputs
    rows = np.arange(B)[:, None]
    np.add.at(final, (rows, np.asarray(ebv)), (1.0 - pg_full) * attn)
    return final, h, c, c_t, attn, pg_full, covn


# revision 14
# speedup vs baseline: 1.1485x; 1.0482x over previous
"""Trainium2 Bass kernel for nn_Decoder_65498251264356.

Pointer-generator decoder step: embedding + LSTM cell + Bahdanau attention
(with coverage) + pointer-gate + vocab softmax + pointer scatter-mix.

Sharding: data-parallel over batch B=128 across 8 cores (16 rows each) for
the LSTM/attention front (launch A); the vocab projection is
tensor-parallel over V=50000 (6250 columns per core, launch B) using the
s2/p_gen rows gathered on the host between launches; the vocab softmax
normalizer is an 8-way partial-sum exchange through the host, applied
on-device in launch C. The final pointer scatter-add is applied on the
host during unsharding, using device-computed attn and p_gen.
(Device collectives compile but fail to load under this axon terminal, so
cross-core exchanges ride the host launch boundaries instead.)
"""
import sys

sys.path.insert(0, "/opt/trn_rl_repo")

import numpy as np
import ml_dtypes

import concourse.bass as bass
import concourse.mybir as mybir
import concourse.tile as tile
from concourse.masks import make_identity
from concourse.vector_clock import ScopedClock

dt = mybir.dt
AF = mybir.ActivationFunctionType
ALU = mybir.AluOpType

B, T, H, E, V = 128, 400, 512, 128, 50000
NCORES = 8
BL = B // NCORES        # 16 batch rows per core
VL = V // NCORES        # 6250 vocab cols per core
H2 = 2 * H              # 1024
EPS = 1e-12
BF = dt.bfloat16
F32 = dt.float32

NCH = [(i * 512, min(512, VL - i * 512)) for i in range((VL + 511) // 512)]


# ---------------------------------------------------------------------------
# walrus in this container rejects >1 sem wait per instruction; split the
# Tile tail-drain's aggregated waits onto single-wait NOPs.
def _patched_drain_and_barrier(self, tick_clock, wait_clock):
    nc = self.nc
    carrier = nc.sync.nop(nofuse=True)
    wait_clock.add_sem_waits(carrier.ins, ScopedClock({None: tick_clock.global_clock}))
    si = carrier.ins.sync_info
    waits = list(si.on_wait or []) if si else []
    if len(waits) > 1:
        carrier.ins.sync_info = mybir.SyncInfo(
            on_wait=waits[:1], on_update=list(si.on_update or [])
        )
        for w in waits[1:]:
            n = nc.sync.nop(nofuse=True)
            n.ins.sync_info = mybir.SyncInfo(on_wait=[w], on_update=[])
    nc.sync.drain()
    nc.all_engine_barrier()
    assert self.sems is not None
    popped = nc._tile_sem_poison_stack.pop()
    assert popped is self._sem_poison
    nc.clear_and_free_semaphores(list(self.sems.allocated().values()))
    nc.all_engine_barrier()


tile.TileContext._drain_and_barrier = _patched_drain_and_barrier

SPLIT_WAITS = True
_wsplit_ctr = [0]


def _split_multi_waits(nc):
    if not SPLIT_WAITS:
        return
    """Same walrus limit, applied globally: extra waits move onto
    single-wait NOPs inserted just before the instruction, same engine."""
    for f in nc.m.functions:
        for bb in f.blocks:
            il = bb.instructions
            i = 0
            while i < len(il):
                inst = il[i]
                si = inst.sync_info
                waits = list(si.on_wait) if si and si.on_wait else []
                if len(waits) > 1:
                    for w in waits[:-1]:
                        _wsplit_ctr[0] += 1
                        nop = mybir.InstNoOp(
                            name=f"I-wsplit-{_wsplit_ctr[0]}",
                            engine=inst.engine,
                            sync_info=mybir.SyncInfo(on_wait=[w], on_update=[]),
                        )
                        il.insert(i, nop)
                        i += 1
                    inst.sync_info = mybir.SyncInfo(
                        on_wait=[waits[-1]], on_update=list(si.on_update or [])
                    )
                i += 1
# ---------------------------------------------------------------------------


def build_program_a(use_cov: bool):
    """LSTM + attention + p_gen + s2, data-parallel over 16 batch rows."""
    nc = bass.Bass()

    xinT = nc.dram_tensor("xinT", [9 * E, BL], F32, kind="ExternalInput")   # [c_t_1; emb].T
    h0T = nc.dram_tensor("h0T", [H, BL], F32, kind="ExternalInput")
    c0T = nc.dram_tensor("c0T", [H, BL], F32, kind="ExternalInput")
    encfT = nc.dram_tensor("encfT", [BL, H2, T], BF, kind="ExternalInput")  # feature-major
    enco = nc.dram_tensor("enco", [BL, T, H2], BF, kind="ExternalInput")    # natural
    mask = nc.dram_tensor("mask", [BL, T], F32, kind="ExternalInput")
    cov = nc.dram_tensor("cov", [BL, T], F32, kind="ExternalInput")
    Wxc = nc.dram_tensor("Wxc", [9 * E, E], F32, kind="ExternalInput")      # W_xc.T
    bxc = nc.dram_tensor("bxc", [E, 1], F32, kind="ExternalInput")
    Wih = nc.dram_tensor("Wih", [E, 4 * H], F32, kind="ExternalInput")      # W_ih.T
    Whh = nc.dram_tensor("Whh", [H, 4 * H], F32, kind="ExternalInput")      # W_hh.T
    bih2 = nc.dram_tensor("bih2", [128, 16], F32, kind="ExternalInput")
    bhh2 = nc.dram_tensor("bhh2", [128, 16], F32, kind="ExternalInput")
    Wdp = nc.dram_tensor("Wdp", [H2, H2], F32, kind="ExternalInput")        # W_dp.T
    bdp2 = nc.dram_tensor("bdp2", [128, 8], F32, kind="ExternalInput")
    vw2 = nc.dram_tensor("vw2", [128, 8], BF, kind="ExternalInput")         # v chunks
    wcrow = nc.dram_tensor("wcrow", [1, H2], BF, kind="ExternalInput")      # W_c row
    covrow = nc.dram_tensor("covrow", [1, BL * T], BF, kind="ExternalInput")
    Wpg = nc.dram_tensor("Wpg", [128, 17], F32, kind="ExternalInput")       # W_pg.T chunks
    bpg = nc.dram_tensor("bpg", [1, 1], F32, kind="ExternalInput")
    Wo1 = nc.dram_tensor("Wo1", [3 * H, H], F32, kind="ExternalInput")      # W_o1.T
    bo12 = nc.dram_tensor("bo12", [128, 4], F32, kind="ExternalInput")

    h_out = nc.dram_tensor("h_out", [H, BL], F32, kind="ExternalOutput")
    c_out = nc.dram_tensor("c_out", [H, BL], F32, kind="ExternalOutput")
    ct_out = nc.dram_tensor("ct_out", [H2, BL], F32, kind="ExternalOutput")
    attn_out = nc.dram_tensor("attn_out", [BL, T], F32, kind="ExternalOutput")
    pg_out = nc.dram_tensor("pg_out", [BL, 1], F32, kind="ExternalOutput")
    covn_out = nc.dram_tensor("covn_out", [BL, T], F32, kind="ExternalOutput")
    s2_out = nc.dram_tensor("s2_out", [H, BL], F32, kind="ExternalOutput")

    with tile.TileContext(nc) as tc:
        with (
            tc.tile_pool(name="wp", bufs=1) as wp,
            tc.tile_pool(name="fp", bufs=1) as fp,
            tc.tile_pool(name="ap", bufs=3) as ap,
            tc.tile_pool(name="pss", bufs=2, space="PSUM") as pss,
            tc.tile_pool(name="pssc", bufs=2, space="PSUM") as pssc,
            tc.tile_pool(name="psct", bufs=1, space="PSUM") as psct,
        ):
            # ---- persistent small tiles ----
            vw_t = wp.tile([128, 8], BF)
            nc.sync.dma_start(out=vw_t[:], in_=vw2[:])
            bxc_t = wp.tile([128, 1], F32)
            nc.sync.dma_start(out=bxc_t[:], in_=bxc[:])
            bih_t = wp.tile([128, 16], F32)
            nc.sync.dma_start(out=bih_t[:], in_=bih2[:])
            bhh_t = wp.tile([128, 16], F32)
            nc.sync.dma_start(out=bhh_t[:], in_=bhh2[:])
            bdp_t = wp.tile([128, 8], F32)
            nc.sync.dma_start(out=bdp_t[:], in_=bdp2[:])
            bo1_t = wp.tile([128, 4], F32)
            nc.sync.dma_start(out=bo1_t[:], in_=bo12[:])
            bpg_t = wp.tile([1, 1], F32)
            nc.sync.dma_start(out=bpg_t[:], in_=bpg[:])
            wpg_t = wp.tile([128, 17], F32)
            nc.sync.dma_start(out=wpg_t[:], in_=Wpg[:])
            mask_t = wp.tile([BL, T], F32)
            nc.sync.dma_start(out=mask_t[:], in_=mask[:])
            cov_t = wp.tile([BL, T], F32)
            nc.sync.dma_start(out=cov_t[:], in_=cov[:])
            if use_cov:
                wcrow_t = wp.tile([1, H2], BF)
                nc.sync.dma_start(out=wcrow_t[:], in_=wcrow[:])
                covrow_t = wp.tile([1, BL * T], BF)
                nc.sync.dma_start(out=covrow_t[:], in_=covrow[:])
            ident = wp.tile([128, 128], F32)
            make_identity(nc, ident[:])

            # combined LSTM gate bias, plus halved for sigmoid-via-tanh:
            # sigmoid(x) = 0.5*tanh(0.5*x) + 0.5
            bg_t = wp.tile([128, 16], F32)
            nc.vector.tensor_tensor(out=bg_t[:], in0=bih_t[:], in1=bhh_t[:], op=ALU.add)
            bgh_t = wp.tile([128, 16], F32)
            nc.vector.tensor_scalar_mul(out=bgh_t[:], in0=bg_t[:], scalar1=0.5)
            bpgh_t = wp.tile([1, 1], F32)
            nc.vector.tensor_scalar_mul(out=bpgh_t[:], in0=bpg_t[:], scalar1=0.5)

            # ---- big front weights (recycled slots) ----
            wxc_t = fp.tile([128, 9, E], F32, tag="wxc")
            nc.gpsimd.dma_start(out=wxc_t[:], in_=Wxc[:].rearrange("(c p) m -> p c m", p=128))
            wih_t = fp.tile([128, 4 * H], F32, tag="wih")
            nc.gpsimd.dma_start(out=wih_t[:], in_=Wih[:])
            whh_t = fp.tile([128, 4, 4 * H], F32, tag="whh")
            nc.gpsimd.dma_start(out=whh_t[:], in_=Whh[:].rearrange("(c p) m -> p c m", p=128))
            wdp_t = fp.tile([128, 8, H2], F32, tag="wdp")
            nc.gpsimd.dma_start(out=wdp_t[:], in_=Wdp[:].rearrange("(c p) m -> p c m", p=128))
            wo1_t = fp.tile([128, 12, H], F32, tag="wo1")
            nc.gpsimd.dma_start(out=wo1_t[:], in_=Wo1[:].rearrange("(c p) m -> p c m", p=128))

            # ---- x = [c_t_1; emb] @ W_xc.T + b ----
            xin_t = fp.tile([128, 9, BL], F32, tag="xin")
            nc.sync.dma_start(out=xin_t[:], in_=xinT[:].rearrange("(c p) b -> p c b", p=128))
            h0_t = fp.tile([128, 4, BL], F32, tag="h0")
            nc.sync.dma_start(out=h0_t[:], in_=h0T[:].rearrange("(c p) b -> p c b", p=128))
            c0_t = fp.tile([128, 4, BL], F32, tag="c0")
            nc.sync.dma_start(out=c0_t[:], in_=c0T[:].rearrange("(c p) b -> p c b", p=128))

            ps_x = pss.tile([128, BL], F32, space="PSUM", tag="ps_small")
            for c in range(9):
                nc.tensor.matmul(out=ps_x[:], lhsT=wxc_t[:, c, :], rhs=xin_t[:, c, :],
                                 start=(c == 0), stop=(c == 8))
            xs = fp.tile([128, BL], F32, tag="xs")
            nc.vector.tensor_scalar_add(out=xs[:], in0=ps_x[:], scalar1=bxc_t[:, 0:1])

            # ---- LSTM gates (order i,f,g,o) ----
            i_s = fp.tile([128, 4, BL], F32, tag="i_s")
            f_s = fp.tile([128, 4, BL], F32, tag="f_s")
            g_s = fp.tile([128, 4, BL], F32, tag="g_s")
            o_s = fp.tile([128, 4, BL], F32, tag="o_s")
            gate_dst = [i_s, f_s, g_s, o_s]
            for mc in range(16):
                ps_g = pss.tile([128, BL], F32, space="PSUM", tag="ps_small")
                nc.tensor.matmul(out=ps_g[:], lhsT=wih_t[:, mc * 128:(mc + 1) * 128],
                                 rhs=xs[:], start=True, stop=False)
                for c in range(4):
                    nc.tensor.matmul(out=ps_g[:], lhsT=whh_t[:, c, mc * 128:(mc + 1) * 128],
                                     rhs=h0_t[:, c, :], start=False, stop=(c == 3))
                dst = gate_dst[mc // 4][:, mc % 4, :]
                if mc // 4 == 2:  # g -> tanh(x + b)
                    nc.scalar.activation(out=dst, in_=ps_g[:], func=AF.Tanh,
                                         bias=bg_t[:, mc:mc + 1], scale=1.0)
                else:  # i,f,o -> sigmoid via tanh
                    nc.scalar.activation(out=dst, in_=ps_g[:], func=AF.Tanh,
                                         bias=bgh_t[:, mc:mc + 1], scale=0.5)
            for gidx in (0, 1, 3):
                g = gate_dst[gidx]
                nc.vector.tensor_scalar(out=g[:], in0=g[:], scalar1=0.5, scalar2=0.5,
                                        op0=ALU.mult, op1=ALU.add)

            # ---- c, h ----
            cT_s = fp.tile([128, 4, BL], F32, tag="cT")
            hT_s = fp.tile([128, 4, BL], F32, tag="hT")
            tnc = fp.tile([128, 4, BL], F32, tag="tnc")
            for c in range(4):
                t1 = fp.tile([128, BL], F32, tag="lstm_t1")
                nc.vector.tensor_tensor(out=t1[:], in0=f_s[:, c, :], in1=c0_t[:, c, :], op=ALU.mult)
                t2 = fp.tile([128, BL], F32, tag="lstm_t2")
                nc.vector.tensor_tensor(out=t2[:], in0=i_s[:, c, :], in1=g_s[:, c, :], op=ALU.mult)
                nc.vector.tensor_tensor(out=cT_s[:, c, :], in0=t1[:], in1=t2[:], op=ALU.add)
                nc.scalar.activation(out=tnc[:, c, :], in_=cT_s[:, c, :], func=AF.Tanh)
                nc.vector.tensor_tensor(out=hT_s[:, c, :], in0=o_s[:, c, :], in1=tnc[:, c, :], op=ALU.mult)

            nc.sync.dma_start(out=h_out[:].rearrange("(c p) b -> p c b", p=128), in_=hT_s[:])
            nc.sync.dma_start(out=c_out[:].rearrange("(c p) b -> p c b", p=128), in_=cT_s[:])

            # ---- dec_fea = s_t_hat @ W_dp.T + b_dp ----
            dec_t = fp.tile([128, 8, BL], F32, tag="dec")
            for mc in range(8):
                ps_d = pss.tile([128, BL], F32, space="PSUM", tag="ps_small")
                for c in range(8):
                    rhs = hT_s[:, c, :] if c < 4 else cT_s[:, c - 4, :]
                    nc.tensor.matmul(out=ps_d[:], lhsT=wdp_t[:, c, mc * 128:(mc + 1) * 128],
                                     rhs=rhs, start=(c == 0), stop=(c == 7))
                nc.vector.tensor_scalar_add(out=dec_t[:, mc, :], in0=ps_d[:],
                                            scalar1=bdp_t[:, mc:mc + 1])

            # ---- attention pass 1: scores -> softmax ----
            # (engines may only address partition strips at 0/32/64/96, so
            # per-b exp results land on partition 0 as column slices and are
            # redistributed to [BL, T] with one SBUF->SBUF DMA)
            expsc_row = fp.tile([1, BL * T], F32, tag="expsc_row")
            ssum_row = fp.tile([1, BL], F32, tag="ssum_row")
            for b in range(BL):
                eF = ap.tile([128, 8, T], BF, tag="eF")
                nc.sync.dma_start(out=eF[:], in_=encfT[b].rearrange("(c p) t -> p c t", p=128))
                eE = ap.tile([128, 8, T], BF, tag="eE")
                if use_cov:
                    for c in range(8):
                        ps_cov = pssc.tile([128, T], F32, space="PSUM", tag="ps_cov")
                        nc.tensor.matmul(out=ps_cov[:], lhsT=wcrow_t[:, c * 128:(c + 1) * 128],
                                         rhs=covrow_t[:, b * T:(b + 1) * T], start=True, stop=True)
                        t3 = ap.tile([128, T], F32, tag="covtmp")
                        nc.vector.tensor_scalar_add(out=t3[:], in0=eF[:, c, :],
                                                    scalar1=dec_t[:, c, b:b + 1])
                        nc.vector.tensor_tensor(out=eE[:, c, :], in0=t3[:], in1=ps_cov[:], op=ALU.add)
                else:
                    for c in range(8):
                        nc.vector.tensor_scalar_add(out=eE[:, c, :], in0=eF[:, c, :],
                                                    scalar1=dec_t[:, c, b:b + 1])
                nc.scalar.activation(out=eE[:], in_=eE[:], func=AF.Tanh)
                ps_sc = pssc.tile([1, T], F32, space="PSUM", tag="ps_sc")
                for c in range(8):
                    nc.tensor.matmul(out=ps_sc[:], lhsT=vw_t[:, c:c + 1], rhs=eE[:, c, :],
                                     start=(c == 0), stop=(c == 7))
                nc.scalar.activation(out=expsc_row[:, b * T:(b + 1) * T], in_=ps_sc[:],
                                     func=AF.Exp, accum_out=ssum_row[:, b:b + 1])

            expsc = fp.tile([BL, T], F32, tag="expsc")
            nc.sync.dma_start(out=expsc[:], in_=expsc_row[:].rearrange("x (b t) -> x b t", b=BL))
            ssum = fp.tile([BL, 1], F32, tag="ssum")
            nc.sync.dma_start(out=ssum[:], in_=ssum_row[:].rearrange("x (b o) -> x b o", b=BL))

            # softmax tail; equals attn_/(sum(attn_)+eps), attn_ = softmax*mask
            m1 = fp.tile([BL, T], F32, tag="m1")
            nc.vector.tensor_tensor(out=m1[:], in0=expsc[:], in1=mask_t[:], op=ALU.mult)
            s1 = fp.tile([BL, 1], F32, tag="s1")
            nc.vector.reduce_sum(out=s1[:], in_=m1[:], axis=mybir.AxisListType.X)
            den = fp.tile([BL, 1], F32, tag="den")
            nc.vector.tensor_scalar_mul(out=den[:], in0=ssum[:], scalar1=float(EPS))
            nc.vector.tensor_tensor(out=den[:], in0=den[:], in1=s1[:], op=ALU.add)
            rden = fp.tile([BL, 1], F32, tag="rden")
            nc.vector.reciprocal(out=rden[:], in_=den[:])
            attn_t = fp.tile([BL, T], F32, tag="attn")
            nc.vector.tensor_scalar_mul(out=attn_t[:], in0=m1[:], scalar1=rden[:, 0:1])
            nc.sync.dma_start(out=attn_out[:], in_=attn_t[:])
            covn_t = fp.tile([BL, T], F32, tag="covn")
            nc.vector.tensor_tensor(out=covn_t[:], in0=cov_t[:], in1=attn_t[:], op=ALU.add)
            nc.sync.dma_start(out=covn_out[:], in_=covn_t[:])

            # attn.T chunks for c_t matmuls (PE transpose), bf16.
            # T=400 wraps as 4 chunks of 100 partitions so enco[b] loads in
            # ONE DMA below (no ragged 16-row tail transfer).
            attnT_s = fp.tile([100, 4, BL], BF, tag="attnT")
            for q in range(4):
                lo = q * 100
                ps_tr = pss.tile([128, BL], F32, space="PSUM", tag="ps_small")
                nc.tensor.transpose(out=ps_tr[:100, :], in_=attn_t[:, lo:lo + 100],
                                    identity=ident[:BL, :BL])
                nc.vector.tensor_copy(out=attnT_s[:, q, :], in_=ps_tr[:100, :])

            # ---- attention pass 2: c_t ----
            ps_ct = psct.tile([128, 8 * BL], F32, space="PSUM")
            for b in range(BL):
                oT = ap.tile([100, 4, H2], BF, tag="oT")
                nc.gpsimd.dma_start(out=oT[:], in_=enco[b].rearrange("(q p) f -> p q f", p=100))
                for fc in range(8):
                    col = fc * BL + b
                    for q in range(4):
                        nc.tensor.matmul(out=ps_ct[:, col:col + 1],
                                         lhsT=oT[:, q, fc * 128:(fc + 1) * 128],
                                         rhs=attnT_s[:, q, b:b + 1],
                                         start=(q == 0), stop=(q == 3))
            ct_s = fp.tile([128, 8, BL], F32, tag="ct")
            nc.vector.tensor_copy(out=ct_s[:], in_=ps_ct[:].rearrange("p (fc b) -> p fc b", fc=8))
            nc.sync.dma_start(out=ct_out[:].rearrange("(fc p) b -> p fc b", p=128), in_=ct_s[:])

            # ---- p_gen ----
            ps_pg = pss.tile([1, BL], F32, space="PSUM", tag="ps_small")
            pg_rhs = [ct_s[:, k, :] for k in range(8)] + \
                     [hT_s[:, k, :] for k in range(4)] + \
                     [cT_s[:, k, :] for k in range(4)] + [xs[:]]
            for k in range(17):
                nc.tensor.matmul(out=ps_pg[:], lhsT=wpg_t[:, k:k + 1], rhs=pg_rhs[k],
                                 start=(k == 0), stop=(k == 16))
            pg_s = fp.tile([1, BL], F32, tag="pg")
            nc.scalar.activation(out=pg_s[:], in_=ps_pg[:], func=AF.Tanh,
                                 bias=bpgh_t[:, 0:1], scale=0.5)
            nc.vector.tensor_scalar(out=pg_s[:], in0=pg_s[:], scalar1=0.5, scalar2=0.5,
                                    op0=ALU.mult, op1=ALU.add)
            nc.sync.dma_start(out=pg_out[:].rearrange("b x -> x b"), in_=pg_s[:])

            # ---- s2 = [h; c_t] @ W_o1.T + b_o1 ----
            s2_s = fp.tile([128, 4, BL], F32, tag="s2")
            for mc in range(4):
                ps_s2 = pss.tile([128, BL], F32, space="PSUM", tag="ps_small")
                for c in range(12):
                    rhs = hT_s[:, c, :] if c < 4 else ct_s[:, c - 4, :]
                    nc.tensor.matmul(out=ps_s2[:], lhsT=wo1_t[:, c, mc * 128:(mc + 1) * 128],
                                     rhs=rhs, start=(c == 0), stop=(c == 11))
                nc.vector.tensor_scalar_add(out=s2_s[:, mc, :], in0=ps_s2[:],
                                            scalar1=bo1_t[:, mc:mc + 1])
            nc.sync.dma_start(out=s2_out[:].rearrange("(c p) b -> p c b", p=128), in_=s2_s[:])

    _split_multi_waits(nc)
    return nc


def build_program_b():
    """Vocab projection + exp, tensor-parallel over 6250 vocab columns."""
    nc = bass.Bass()
    s2gT = nc.dram_tensor("s2gT", [4, 128, B], BF, kind="ExternalInput")   # s2_full.T chunks
    Wo2 = nc.dram_tensor("Wo2", [4, 128, VL], BF, kind="ExternalInput")    # W_o2.T chunks
    bo2 = nc.dram_tensor("bo2", [1, VL], BF, kind="ExternalInput")
    expv_out = nc.dram_tensor("expv_out", [B, VL], F32, kind="ExternalOutput")
    vsum_out = nc.dram_tensor("vsum_out", [B, 1], F32, kind="ExternalOutput")

    with tile.TileContext(nc) as tc:
        with (
            tc.tile_pool(name="wp", bufs=1) as wp,
            tc.tile_pool(name="wo2s", bufs=4) as wo2p,
            tc.tile_pool(name="psb", bufs=2, space="PSUM") as psb,
        ):
            s2g = wp.tile([128, 4, B], BF)
            nc.sync.dma_start(out=s2g[:], in_=s2gT[:].rearrange("c p b -> p c b"))
            bo2_t = wp.tile([1, VL], BF)
            nc.sync.dma_start(out=bo2_t[:], in_=bo2[:])
            ones1 = wp.tile([1, 128], BF)
            nc.vector.memset(ones1[:], 1.0)
            expv = wp.tile([128, VL], F32)
            vsum = wp.tile([128, len(NCH)], F32)
            for i, (lo, w) in enumerate(NCH):
                wo2c = wo2p.tile([128, 4, 512], BF, tag="wo2c")
                eng = nc.sync if i % 2 == 0 else nc.gpsimd
                eng.dma_start(out=wo2c[:, :, :w],
                              in_=Wo2[:, :, lo:lo + w].rearrange("c p n -> p c n"))
                ps_o = psb.tile([128, 512], F32, space="PSUM", tag="ps_o")
                for kc in range(4):
                    nc.tensor.matmul(out=ps_o[:, :w], lhsT=s2g[:, kc, :],
                                     rhs=wo2c[:, kc, :w],
                                     start=(kc == 0), stop=False)
                nc.tensor.matmul(out=ps_o[:, :w], lhsT=ones1[:], rhs=bo2_t[:, lo:lo + w],
                                 start=False, stop=True)
                nc.scalar.activation(out=expv[:, lo:lo + w], in_=ps_o[:, :w], func=AF.Exp,
                                     accum_out=vsum[:, i:i + 1])
            vsum_t = wp.tile([128, 1], F32)
            nc.vector.reduce_sum(out=vsum_t[:], in_=vsum[:], axis=mybir.AxisListType.X)
            nc.sync.dma_start(out=vsum_out[:], in_=vsum_t[:])
            nc.sync.dma_start(out=expv_out[:], in_=expv[:])

    _split_multi_waits(nc)
    return nc


def build_program_c():
    """final_cols = expv * (p_gen / S) — the cross-core-normalized scale."""
    nc = bass.Bass()
    expv_in = nc.dram_tensor("expv_in", [B, VL], F32, kind="ExternalInput")
    scale_in = nc.dram_tensor("scale_in", [B, 1], F32, kind="ExternalInput")
    final_cols = nc.dram_tensor("final_cols", [B, VL], F32, kind="ExternalOutput")
    with tile.TileContext(nc) as tc:
        with tc.tile_pool(name="p", bufs=1) as p:
            ev = p.tile([128, VL], F32)
            nc.gpsimd.dma_start(out=ev[:], in_=expv_in[:])
            sc = p.tile([128, 1], F32)
            nc.sync.dma_start(out=sc[:], in_=scale_in[:])
            nc.vector.tensor_scalar_mul(out=ev[:], in0=ev[:], scalar1=sc[:, 0:1])
            nc.sync.dma_start(out=final_cols[:], in_=ev[:])
    _split_multi_waits(nc)
    return nc


# ---------------------------------------------------------------------------
_PROGRAMS: dict = {}


def _get_program(key, builder, *args):
    if key not in _PROGRAMS:
        _PROGRAMS[key] = builder(*args)
    return _PROGRAMS[key]


def _bf(x):
    return np.asarray(x, np.float32).astype(ml_dtypes.bfloat16)


def prep_in_maps(y_t_1, h0, c0, c_t_1, encoder_outputs, encoder_feature, mask_select,
                 enc_batch_extend_vocab, coverage, emb_table, W_c, W_dp, b_dp, v_w,
                 W_xc, b_xc, W_ih, W_hh, b_ih, b_hh, W_pg, b_pg, W_o1, b_o1, W_o2, b_o2):
    to32 = lambda a: np.asarray(a, np.float32)
    y_t_1 = np.asarray(y_t_1)
    h0, c0, c_t_1 = to32(h0), to32(c0), to32(c_t_1)
    encoder_outputs, encoder_feature = to32(encoder_outputs), to32(encoder_feature)
    mask_select, coverage, emb_table = to32(mask_select), to32(coverage), to32(emb_table)
    W_c, W_dp, b_dp, v_w = to32(W_c), to32(W_dp), to32(b_dp), to32(v_w)
    W_xc, b_xc, W_ih, W_hh = to32(W_xc), to32(b_xc), to32(W_ih), to32(W_hh)
    b_ih, b_hh, W_pg, b_pg = to32(b_ih), to32(b_hh), to32(W_pg), to32(b_pg)
    W_o1, b_o1, W_o2, b_o2 = to32(W_o1), to32(b_o1), to32(W_o2), to32(b_o2)

    y_emb = emb_table[y_t_1]                                   # [B, E]
    shared = dict(
        Wxc=np.ascontiguousarray(W_xc.T),
        bxc=b_xc.reshape(E, 1),
        Wih=np.ascontiguousarray(W_ih.T),
        Whh=np.ascontiguousarray(W_hh.T),
        bih2=np.ascontiguousarray(b_ih.reshape(16, 128).T),
        bhh2=np.ascontiguousarray(b_hh.reshape(16, 128).T),
        Wdp=np.ascontiguousarray(W_dp.T),
        bdp2=np.ascontiguousarray(b_dp.reshape(8, 128).T),
        vw2=_bf(v_w[0].reshape(8, 128).T),
        wcrow=_bf(W_c[:, 0].reshape(1, H2)),
        Wpg=np.ascontiguousarray(W_pg[0].reshape(17, 128).T),
        bpg=b_pg.reshape(1, 1),
        Wo1=np.ascontiguousarray(W_o1.T),
        bo12=np.ascontiguousarray(b_o1.reshape(4, 128).T),
    )
    in_maps = []
    for k in range(NCORES):
        bsl = slice(k * BL, (k + 1) * BL)
        m = dict(shared)
        m["xinT"] = np.ascontiguousarray(
            np.concatenate([c_t_1[bsl], y_emb[bsl]], axis=1).T)
        m["h0T"] = np.ascontiguousarray(h0[bsl].T)
        m["c0T"] = np.ascontiguousarray(c0[bsl].T)
        m["encfT"] = _bf(np.ascontiguousarray(encoder_feature[bsl].transpose(0, 2, 1)))
        m["enco"] = _bf(encoder_outputs[bsl])
        m["mask"] = np.ascontiguousarray(mask_select[bsl])
        m["cov"] = np.ascontiguousarray(coverage[bsl])
        m["covrow"] = _bf(coverage[bsl].reshape(1, BL * T))
        in_maps.append(m)

    Wo2T_bf = _bf(np.ascontiguousarray(W_o2.T)).reshape(4, 128, V)
    bo2_bf = _bf(b_o2.reshape(1, V))
    b_maps_wo2 = []
    for k in range(NCORES):
        vsl = slice(k * VL, (k + 1) * VL)
        b_maps_wo2.append(dict(Wo2=np.ascontiguousarray(Wo2T_bf[:, :, vsl]),
                               bo2=np.ascontiguousarray(bo2_bf[:, vsl])))
    use_cov = bool(np.any(coverage != 0.0))
    return in_maps, b_maps_wo2, use_cov, enc_batch_extend_vocab


def _run_spmd(nc, in_maps):
    from concourse.bass_utils import run_bass_kernel_spmd
    return run_bass_kernel_spmd(nc, in_maps, list(range(NCORES))).results


RUN_WALL = {}


def kernel(**inputs):
    import time as _time
    in_maps, b_maps_wo2, use_cov, ebv = prep_in_maps(**inputs)
    nc_a = _get_program(("a", use_cov), build_program_a, use_cov)
    nc_b = _get_program("b", build_program_b)
    nc_c = _get_program("c", build_program_c)

    t0 = _time.time()
    res_a = _run_spmd(nc_a, in_maps)
    RUN_WALL["a"] = _time.time() - t0

    s2_full = np.concatenate([np.asarray(res_a[k]["s2_out"]).T for k in range(NCORES)], 0)
    pg_full = np.concatenate([np.asarray(res_a[k]["pg_out"]) for k in range(NCORES)], 0)
    s2gT = _bf(np.stack([s2_full[:, kc * 128:(kc + 1) * 128].T for kc in range(4)], 0))
    b_in = [dict(s2gT=s2gT, **b_maps_wo2[k]) for k in range(NCORES)]

    t0 = _time.time()
    res_b = _run_spmd(nc_b, b_in)
    RUN_WALL["b"] = _time.time() - t0

    S = np.sum([np.asarray(res_b[k]["vsum_out"]) for k in range(NCORES)], axis=0)  # [B,1]
    scale = (pg_full / S).astype(np.float32)
    c_in = [dict(expv_in=np.asarray(res_b[k]["expv_out"]), scale_in=scale)
            for k in range(NCORES)]

    t0 = _time.time()
    res_c = _run_spmd(nc_c, c_in)
    RUN_WALL["c"] = _time.time() - t0

    final = np.concatenate([np.asarray(res_c[k]["final_cols"]) for k in range(NCORES)], 1)
    h = np.concatenate([np.asarray(res_a[k]["h_out"]).T for k in range(NCORES)], 0)
    c = np.concatenate([np.asarray(res_a[k]["c_out"]).T for k in range(NCORES)], 0)
    c_t = np.concatenate([np.asarray(res_a[k]["ct_out"]).T for k in range(NCORES)], 0)
    attn = np.concatenate([np.asarray(res_a[k]["attn_out"]) for k in range(NCORES)], 0)
    covn = np.concatenate([np.asarray(res_a[k]["covn_out"]) for k in range(NCORES)], 0)

    # pointer scatter-add applied during unshard (host), from device outputs
    rows = np.arange(B)[:, None]
    np.add.at(final, (rows, np.asarray(ebv)), (1.0 - pg_full) * attn)
    return final, h, c, c_t, attn, pg_full, covn


# revision 15
# speedup vs baseline: 1.2914x; 1.1244x over previous
"""Trainium2 Bass kernel for nn_Decoder_65498251264356.

Pointer-generator decoder step: embedding + LSTM cell + Bahdanau attention
(with coverage) + pointer-gate + vocab softmax + pointer scatter-mix.

Sharding: data-parallel over batch B=128 across 8 cores (16 rows each) for
the LSTM/attention front (launch A); the vocab projection is
tensor-parallel over V=50000 (6250 columns per core, launch B) using the
s2/p_gen rows gathered on the host between launches; the vocab softmax
normalizer is an 8-way partial-sum exchange through the host, applied
on-device in launch C. The final pointer scatter-add is applied on the
host during unsharding, using device-computed attn and p_gen.
(Device collectives compile but fail to load under this axon terminal, so
cross-core exchanges ride the host launch boundaries instead.)
"""
import sys

sys.path.insert(0, "/opt/trn_rl_repo")

import numpy as np
import ml_dtypes

import concourse.bass as bass
import concourse.mybir as mybir
import concourse.tile as tile
from concourse.masks import make_identity
from concourse.vector_clock import ScopedClock

dt = mybir.dt
AF = mybir.ActivationFunctionType
ALU = mybir.AluOpType

B, T, H, E, V = 128, 400, 512, 128, 50000
NCORES = 8
BL = B // NCORES        # 16 batch rows per core
VL = V // NCORES        # 6250 vocab cols per core
H2 = 2 * H              # 1024
EPS = 1e-12
BF = dt.bfloat16
F32 = dt.float32

NCH = [(i * 512, min(512, VL - i * 512)) for i in range((VL + 511) // 512)]


# ---------------------------------------------------------------------------
# walrus in this container rejects >1 sem wait per instruction; split the
# Tile tail-drain's aggregated waits onto single-wait NOPs.
def _patched_drain_and_barrier(self, tick_clock, wait_clock):
    nc = self.nc
    carrier = nc.sync.nop(nofuse=True)
    wait_clock.add_sem_waits(carrier.ins, ScopedClock({None: tick_clock.global_clock}))
    si = carrier.ins.sync_info
    waits = list(si.on_wait or []) if si else []
    if len(waits) > 1:
        carrier.ins.sync_info = mybir.SyncInfo(
            on_wait=waits[:1], on_update=list(si.on_update or [])
        )
        for w in waits[1:]:
            n = nc.sync.nop(nofuse=True)
            n.ins.sync_info = mybir.SyncInfo(on_wait=[w], on_update=[])
    nc.sync.drain()
    nc.all_engine_barrier()
    assert self.sems is not None
    popped = nc._tile_sem_poison_stack.pop()
    assert popped is self._sem_poison
    nc.clear_and_free_semaphores(list(self.sems.allocated().values()))
    nc.all_engine_barrier()


tile.TileContext._drain_and_barrier = _patched_drain_and_barrier

SPLIT_WAITS = True
_wsplit_ctr = [0]


def _split_multi_waits(nc):
    if not SPLIT_WAITS:
        return
    """Same walrus limit, applied globally: extra waits move onto
    single-wait NOPs inserted just before the instruction, same engine."""
    for f in nc.m.functions:
        for bb in f.blocks:
            il = bb.instructions
            i = 0
            while i < len(il):
                inst = il[i]
                si = inst.sync_info
                waits = list(si.on_wait) if si and si.on_wait else []
                if len(waits) > 1:
                    for w in waits[:-1]:
                        _wsplit_ctr[0] += 1
                        nop = mybir.InstNoOp(
                            name=f"I-wsplit-{_wsplit_ctr[0]}",
                            engine=inst.engine,
                            sync_info=mybir.SyncInfo(on_wait=[w], on_update=[]),
                        )
                        il.insert(i, nop)
                        i += 1
                    inst.sync_info = mybir.SyncInfo(
                        on_wait=[waits[-1]], on_update=list(si.on_update or [])
                    )
                i += 1
# ---------------------------------------------------------------------------


def build_program_a(use_cov: bool):
    """LSTM + attention + p_gen + s2, data-parallel over 16 batch rows."""
    nc = bass.Bass()

    xinT = nc.dram_tensor("xinT", [9 * E, BL], F32, kind="ExternalInput")   # [c_t_1; emb].T
    h0T = nc.dram_tensor("h0T", [H, BL], F32, kind="ExternalInput")
    c0T = nc.dram_tensor("c0T", [H, BL], F32, kind="ExternalInput")
    encfT = nc.dram_tensor("encfT", [BL, H2, T], BF, kind="ExternalInput")  # feature-major
    enco = nc.dram_tensor("enco", [BL, T, H2], BF, kind="ExternalInput")    # natural
    mask = nc.dram_tensor("mask", [BL, T], F32, kind="ExternalInput")
    cov = nc.dram_tensor("cov", [BL, T], F32, kind="ExternalInput")
    Wxc = nc.dram_tensor("Wxc", [9 * E, E], F32, kind="ExternalInput")      # W_xc.T
    bxc = nc.dram_tensor("bxc", [E, 1], F32, kind="ExternalInput")
    Wih = nc.dram_tensor("Wih", [E, 4 * H], F32, kind="ExternalInput")      # W_ih.T
    Whh = nc.dram_tensor("Whh", [H, 4 * H], F32, kind="ExternalInput")      # W_hh.T
    bih2 = nc.dram_tensor("bih2", [128, 16], F32, kind="ExternalInput")
    bhh2 = nc.dram_tensor("bhh2", [128, 16], F32, kind="ExternalInput")
    Wdp = nc.dram_tensor("Wdp", [H2, H2], F32, kind="ExternalInput")        # W_dp.T
    bdp2 = nc.dram_tensor("bdp2", [128, 8], F32, kind="ExternalInput")
    vw2 = nc.dram_tensor("vw2", [128, 8], BF, kind="ExternalInput")         # v chunks
    wcrow = nc.dram_tensor("wcrow", [1, H2], BF, kind="ExternalInput")      # W_c row
    covrow = nc.dram_tensor("covrow", [1, BL * T], BF, kind="ExternalInput")
    Wpg = nc.dram_tensor("Wpg", [128, 17], F32, kind="ExternalInput")       # W_pg.T chunks
    bpg = nc.dram_tensor("bpg", [1, 1], F32, kind="ExternalInput")
    Wo1 = nc.dram_tensor("Wo1", [3 * H, H], F32, kind="ExternalInput")      # W_o1.T
    bo12 = nc.dram_tensor("bo12", [128, 4], F32, kind="ExternalInput")

    h_out = nc.dram_tensor("h_out", [H, BL], F32, kind="ExternalOutput")
    c_out = nc.dram_tensor("c_out", [H, BL], F32, kind="ExternalOutput")
    ct_out = nc.dram_tensor("ct_out", [H2, BL], F32, kind="ExternalOutput")
    attn_out = nc.dram_tensor("attn_out", [BL, T], F32, kind="ExternalOutput")
    pg_out = nc.dram_tensor("pg_out", [BL, 1], F32, kind="ExternalOutput")
    covn_out = nc.dram_tensor("covn_out", [BL, T], F32, kind="ExternalOutput")
    s2_out = nc.dram_tensor("s2_out", [H, BL], F32, kind="ExternalOutput")

    with tile.TileContext(nc) as tc:
        with (
            tc.tile_pool(name="wp", bufs=1) as wp,
            tc.tile_pool(name="fp", bufs=1) as fp,
            tc.tile_pool(name="ap", bufs=3) as ap,
            tc.tile_pool(name="pss", bufs=2, space="PSUM") as pss,
            tc.tile_pool(name="pssc", bufs=2, space="PSUM") as pssc,
            tc.tile_pool(name="psct", bufs=1, space="PSUM") as psct,
        ):
            # ---- persistent small tiles ----
            vw_t = wp.tile([128, 8], BF)
            nc.sync.dma_start(out=vw_t[:], in_=vw2[:])
            bxc_t = wp.tile([128, 1], F32)
            nc.sync.dma_start(out=bxc_t[:], in_=bxc[:])
            bih_t = wp.tile([128, 16], F32)
            nc.sync.dma_start(out=bih_t[:], in_=bih2[:])
            bhh_t = wp.tile([128, 16], F32)
            nc.sync.dma_start(out=bhh_t[:], in_=bhh2[:])
            bdp_t = wp.tile([128, 8], F32)
            nc.sync.dma_start(out=bdp_t[:], in_=bdp2[:])
            bo1_t = wp.tile([128, 4], F32)
            nc.sync.dma_start(out=bo1_t[:], in_=bo12[:])
            bpg_t = wp.tile([1, 1], F32)
            nc.sync.dma_start(out=bpg_t[:], in_=bpg[:])
            wpg_t = wp.tile([128, 17], F32)
            nc.sync.dma_start(out=wpg_t[:], in_=Wpg[:])
            mask_t = wp.tile([BL, T], F32)
            nc.sync.dma_start(out=mask_t[:], in_=mask[:])
            cov_t = wp.tile([BL, T], F32)
            nc.sync.dma_start(out=cov_t[:], in_=cov[:])
            if use_cov:
                wcrow_t = wp.tile([1, H2], BF)
                nc.sync.dma_start(out=wcrow_t[:], in_=wcrow[:])
                covrow_t = wp.tile([1, BL * T], BF)
                nc.sync.dma_start(out=covrow_t[:], in_=covrow[:])
            ident = wp.tile([128, 128], F32)
            make_identity(nc, ident[:])

            # combined LSTM gate bias, plus halved for sigmoid-via-tanh:
            # sigmoid(x) = 0.5*tanh(0.5*x) + 0.5
            bg_t = wp.tile([128, 16], F32)
            nc.vector.tensor_tensor(out=bg_t[:], in0=bih_t[:], in1=bhh_t[:], op=ALU.add)
            bgh_t = wp.tile([128, 16], F32)
            nc.vector.tensor_scalar_mul(out=bgh_t[:], in0=bg_t[:], scalar1=0.5)
            bpgh_t = wp.tile([1, 1], F32)
            nc.vector.tensor_scalar_mul(out=bpgh_t[:], in0=bpg_t[:], scalar1=0.5)

            # ---- big front weights (recycled slots) ----
            wxc_t = fp.tile([128, 9, E], F32, tag="wxc")
            nc.gpsimd.dma_start(out=wxc_t[:], in_=Wxc[:].rearrange("(c p) m -> p c m", p=128))
            wih_t = fp.tile([128, 4 * H], F32, tag="wih")
            nc.gpsimd.dma_start(out=wih_t[:], in_=Wih[:])
            whh_t = fp.tile([128, 4, 4 * H], F32, tag="whh")
            nc.sync.dma_start(out=whh_t[:], in_=Whh[:].rearrange("(c p) m -> p c m", p=128))
            wdp_t = fp.tile([128, 8, H2], F32, tag="wdp")
            nc.gpsimd.dma_start(out=wdp_t[:], in_=Wdp[:].rearrange("(c p) m -> p c m", p=128))
            wo1_t = fp.tile([128, 12, H], F32, tag="wo1")
            nc.sync.dma_start(out=wo1_t[:], in_=Wo1[:].rearrange("(c p) m -> p c m", p=128))

            # ---- x = [c_t_1; emb] @ W_xc.T + b ----
            xin_t = fp.tile([128, 9, BL], F32, tag="xin")
            nc.sync.dma_start(out=xin_t[:], in_=xinT[:].rearrange("(c p) b -> p c b", p=128))
            h0_t = fp.tile([128, 4, BL], F32, tag="h0")
            nc.sync.dma_start(out=h0_t[:], in_=h0T[:].rearrange("(c p) b -> p c b", p=128))
            c0_t = fp.tile([128, 4, BL], F32, tag="c0")
            nc.sync.dma_start(out=c0_t[:], in_=c0T[:].rearrange("(c p) b -> p c b", p=128))

            ps_x = pss.tile([128, BL], F32, space="PSUM", tag="ps_small")
            for c in range(9):
                nc.tensor.matmul(out=ps_x[:], lhsT=wxc_t[:, c, :], rhs=xin_t[:, c, :],
                                 start=(c == 0), stop=(c == 8))
            xs = fp.tile([128, BL], F32, tag="xs")
            nc.vector.tensor_scalar_add(out=xs[:], in0=ps_x[:], scalar1=bxc_t[:, 0:1])

            # ---- LSTM gates (order i,f,g,o) ----
            i_s = fp.tile([128, 4, BL], F32, tag="i_s")
            f_s = fp.tile([128, 4, BL], F32, tag="f_s")
            g_s = fp.tile([128, 4, BL], F32, tag="g_s")
            o_s = fp.tile([128, 4, BL], F32, tag="o_s")
            gate_dst = [i_s, f_s, g_s, o_s]
            for mc in range(16):
                ps_g = pss.tile([128, BL], F32, space="PSUM", tag="ps_small")
                nc.tensor.matmul(out=ps_g[:], lhsT=wih_t[:, mc * 128:(mc + 1) * 128],
                                 rhs=xs[:], start=True, stop=False)
                for c in range(4):
                    nc.tensor.matmul(out=ps_g[:], lhsT=whh_t[:, c, mc * 128:(mc + 1) * 128],
                                     rhs=h0_t[:, c, :], start=False, stop=(c == 3))
                dst = gate_dst[mc // 4][:, mc % 4, :]
                if mc // 4 == 2:  # g -> tanh(x + b)
                    nc.scalar.activation(out=dst, in_=ps_g[:], func=AF.Tanh,
                                         bias=bg_t[:, mc:mc + 1], scale=1.0)
                else:  # i,f,o -> sigmoid via tanh
                    nc.scalar.activation(out=dst, in_=ps_g[:], func=AF.Tanh,
                                         bias=bgh_t[:, mc:mc + 1], scale=0.5)
            for gidx in (0, 1, 3):
                g = gate_dst[gidx]
                nc.vector.tensor_scalar(out=g[:], in0=g[:], scalar1=0.5, scalar2=0.5,
                                        op0=ALU.mult, op1=ALU.add)

            # ---- c, h ----
            cT_s = fp.tile([128, 4, BL], F32, tag="cT")
            hT_s = fp.tile([128, 4, BL], F32, tag="hT")
            tnc = fp.tile([128, 4, BL], F32, tag="tnc")
            for c in range(4):
                t1 = fp.tile([128, BL], F32, tag="lstm_t1")
                nc.vector.tensor_tensor(out=t1[:], in0=f_s[:, c, :], in1=c0_t[:, c, :], op=ALU.mult)
                t2 = fp.tile([128, BL], F32, tag="lstm_t2")
                nc.vector.tensor_tensor(out=t2[:], in0=i_s[:, c, :], in1=g_s[:, c, :], op=ALU.mult)
                nc.vector.tensor_tensor(out=cT_s[:, c, :], in0=t1[:], in1=t2[:], op=ALU.add)
                nc.scalar.activation(out=tnc[:, c, :], in_=cT_s[:, c, :], func=AF.Tanh)
                nc.vector.tensor_tensor(out=hT_s[:, c, :], in0=o_s[:, c, :], in1=tnc[:, c, :], op=ALU.mult)

            nc.sync.dma_start(out=h_out[:].rearrange("(c p) b -> p c b", p=128), in_=hT_s[:])
            nc.sync.dma_start(out=c_out[:].rearrange("(c p) b -> p c b", p=128), in_=cT_s[:])

            # ---- dec_fea = s_t_hat @ W_dp.T + b_dp ----
            dec_t = fp.tile([128, 8, BL], F32, tag="dec")
            for mc in range(8):
                ps_d = pss.tile([128, BL], F32, space="PSUM", tag="ps_small")
                for c in range(8):
                    rhs = hT_s[:, c, :] if c < 4 else cT_s[:, c - 4, :]
                    nc.tensor.matmul(out=ps_d[:], lhsT=wdp_t[:, c, mc * 128:(mc + 1) * 128],
                                     rhs=rhs, start=(c == 0), stop=(c == 7))
                nc.vector.tensor_scalar_add(out=dec_t[:, mc, :], in0=ps_d[:],
                                            scalar1=bdp_t[:, mc:mc + 1])

            # ---- attention pass 1: scores -> softmax ----
            # (engines may only address partition strips at 0/32/64/96, so
            # per-b exp results land on partition 0 as column slices and are
            # redistributed to [BL, T] with one SBUF->SBUF DMA)
            sc_row = fp.tile([1, BL * T], F32, tag="sc_row")
            for b in range(BL):
                eF = ap.tile([128, 8, T], BF, tag="eF")
                enge = nc.sync if b % 2 == 0 else nc.gpsimd
                enge.dma_start(out=eF[:], in_=encfT[b].rearrange("(c p) t -> p c t", p=128))
                eE = ap.tile([128, 8, T], BF, tag="eE")
                if use_cov:
                    for c in range(8):
                        ps_cov = pssc.tile([128, T], F32, space="PSUM", tag="ps_cov")
                        nc.tensor.matmul(out=ps_cov[:], lhsT=wcrow_t[:, c * 128:(c + 1) * 128],
                                         rhs=covrow_t[:, b * T:(b + 1) * T], start=True, stop=True)
                        t3 = ap.tile([128, T], F32, tag="covtmp")
                        nc.vector.tensor_scalar_add(out=t3[:], in0=eF[:, c, :],
                                                    scalar1=dec_t[:, c, b:b + 1])
                        nc.vector.tensor_tensor(out=eE[:, c, :], in0=t3[:], in1=ps_cov[:], op=ALU.add)
                else:
                    for c in range(8):
                        nc.vector.tensor_scalar_add(out=eE[:, c, :], in0=eF[:, c, :],
                                                    scalar1=dec_t[:, c, b:b + 1])
                nc.scalar.activation(out=eE[:], in_=eE[:], func=AF.Tanh)
                ps_sc = pssc.tile([1, T], F32, space="PSUM", tag="ps_sc")
                for c in range(8):
                    nc.tensor.matmul(out=ps_sc[:], lhsT=vw_t[:, c:c + 1], rhs=eE[:, c, :],
                                     start=(c == 0), stop=(c == 7))
                nc.vector.tensor_copy(out=sc_row[:, b * T:(b + 1) * T], in_=ps_sc[:])

            sc16 = fp.tile([BL, T], F32, tag="sc16")
            nc.sync.dma_start(out=sc16[:], in_=sc_row[:].rearrange("x (b t) -> x b t", b=BL))
            expsc = fp.tile([BL, T], F32, tag="expsc")
            ssum = fp.tile([BL, 1], F32, tag="ssum")
            nc.scalar.activation(out=expsc[:], in_=sc16[:], func=AF.Exp,
                                 accum_out=ssum[:, 0:1])

            # softmax tail; equals attn_/(sum(attn_)+eps), attn_ = softmax*mask
            m1 = fp.tile([BL, T], F32, tag="m1")
            nc.vector.tensor_tensor(out=m1[:], in0=expsc[:], in1=mask_t[:], op=ALU.mult)
            s1 = fp.tile([BL, 1], F32, tag="s1")
            nc.vector.reduce_sum(out=s1[:], in_=m1[:], axis=mybir.AxisListType.X)
            den = fp.tile([BL, 1], F32, tag="den")
            nc.vector.tensor_scalar_mul(out=den[:], in0=ssum[:], scalar1=float(EPS))
            nc.vector.tensor_tensor(out=den[:], in0=den[:], in1=s1[:], op=ALU.add)
            rden = fp.tile([BL, 1], F32, tag="rden")
            nc.vector.reciprocal(out=rden[:], in_=den[:])
            attn_t = fp.tile([BL, T], F32, tag="attn")
            nc.vector.tensor_scalar_mul(out=attn_t[:], in0=m1[:], scalar1=rden[:, 0:1])
            nc.sync.dma_start(out=attn_out[:], in_=attn_t[:])
            covn_t = fp.tile([BL, T], F32, tag="covn")
            nc.vector.tensor_tensor(out=covn_t[:], in0=cov_t[:], in1=attn_t[:], op=ALU.add)
            nc.sync.dma_start(out=covn_out[:], in_=covn_t[:])

            # attn.T chunks for c_t matmuls (PE transpose), bf16.
            # T=400 wraps as 4 chunks of 100 partitions so enco[b] loads in
            # ONE DMA below (no ragged 16-row tail transfer).
            attnT_s = fp.tile([100, 4, BL], BF, tag="attnT")
            for q in range(4):
                lo = q * 100
                ps_tr = pss.tile([128, BL], F32, space="PSUM", tag="ps_small")
                nc.tensor.transpose(out=ps_tr[:100, :], in_=attn_t[:, lo:lo + 100],
                                    identity=ident[:BL, :BL])
                nc.vector.tensor_copy(out=attnT_s[:, q, :], in_=ps_tr[:100, :])

            # ---- attention pass 2: c_t ----
            ps_ct = psct.tile([128, 8 * BL], F32, space="PSUM")
            for b in range(BL):
                oT = ap.tile([100, 4, H2], BF, tag="oT")
                engo = nc.gpsimd if b % 2 == 0 else nc.sync
                engo.dma_start(out=oT[:], in_=enco[b].rearrange("(q p) f -> p q f", p=100))
                for fc in range(8):
                    col = fc * BL + b
                    for q in range(4):
                        nc.tensor.matmul(out=ps_ct[:, col:col + 1],
                                         lhsT=oT[:, q, fc * 128:(fc + 1) * 128],
                                         rhs=attnT_s[:, q, b:b + 1],
                                         start=(q == 0), stop=(q == 3))
            ct_s = fp.tile([128, 8, BL], F32, tag="ct")
            nc.vector.tensor_copy(out=ct_s[:], in_=ps_ct[:].rearrange("p (fc b) -> p fc b", fc=8))
            nc.sync.dma_start(out=ct_out[:].rearrange("(fc p) b -> p fc b", p=128), in_=ct_s[:])

            # ---- p_gen ----
            ps_pg = pss.tile([1, BL], F32, space="PSUM", tag="ps_small")
            pg_rhs = [ct_s[:, k, :] for k in range(8)] + \
                     [hT_s[:, k, :] for k in range(4)] + \
                     [cT_s[:, k, :] for k in range(4)] + [xs[:]]
            for k in range(17):
                nc.tensor.matmul(out=ps_pg[:], lhsT=wpg_t[:, k:k + 1], rhs=pg_rhs[k],
                                 start=(k == 0), stop=(k == 16))
            pg_s = fp.tile([1, BL], F32, tag="pg")
            nc.scalar.activation(out=pg_s[:], in_=ps_pg[:], func=AF.Tanh,
                                 bias=bpgh_t[:, 0:1], scale=0.5)
            nc.vector.tensor_scalar(out=pg_s[:], in0=pg_s[:], scalar1=0.5, scalar2=0.5,
                                    op0=ALU.mult, op1=ALU.add)
            nc.sync.dma_start(out=pg_out[:].rearrange("b x -> x b"), in_=pg_s[:])

            # ---- s2 = [h; c_t] @ W_o1.T + b_o1 ----
            s2_s = fp.tile([128, 4, BL], F32, tag="s2")
            for mc in range(4):
                ps_s2 = pss.tile([128, BL], F32, space="PSUM", tag="ps_small")
                for c in range(12):
                    rhs = hT_s[:, c, :] if c < 4 else ct_s[:, c - 4, :]
                    nc.tensor.matmul(out=ps_s2[:], lhsT=wo1_t[:, c, mc * 128:(mc + 1) * 128],
                                     rhs=rhs, start=(c == 0), stop=(c == 11))
                nc.vector.tensor_scalar_add(out=s2_s[:, mc, :], in0=ps_s2[:],
                                            scalar1=bo1_t[:, mc:mc + 1])
            nc.sync.dma_start(out=s2_out[:].rearrange("(c p) b -> p c b", p=128), in_=s2_s[:])

    _split_multi_waits(nc)
    return nc


def build_program_b():
    """Vocab projection + exp, tensor-parallel over 6250 vocab columns."""
    nc = bass.Bass()
    s2gT = nc.dram_tensor("s2gT", [4, 128, B], BF, kind="ExternalInput")   # s2_full.T chunks
    Wo2 = nc.dram_tensor("Wo2", [4, 128, VL], BF, kind="ExternalInput")    # W_o2.T chunks
    bo2 = nc.dram_tensor("bo2", [1, VL], BF, kind="ExternalInput")
    expv_out = nc.dram_tensor("expv_out", [B, VL], F32, kind="ExternalOutput")
    vsum_out = nc.dram_tensor("vsum_out", [B, 1], F32, kind="ExternalOutput")

    with tile.TileContext(nc) as tc:
        with (
            tc.tile_pool(name="wp", bufs=1) as wp,
            tc.tile_pool(name="wo2s", bufs=4) as wo2p,
            tc.tile_pool(name="psb", bufs=2, space="PSUM") as psb,
        ):
            s2g = wp.tile([128, 4, B], BF)
            nc.sync.dma_start(out=s2g[:], in_=s2gT[:].rearrange("c p b -> p c b"))
            bo2_t = wp.tile([1, VL], BF)
            nc.sync.dma_start(out=bo2_t[:], in_=bo2[:])
            ones1 = wp.tile([1, 128], BF)
            nc.vector.memset(ones1[:], 1.0)
            expv = wp.tile([128, VL], F32)
            vsum = wp.tile([128, len(NCH)], F32)
            for i, (lo, w) in enumerate(NCH):
                wo2c = wo2p.tile([128, 4, 512], BF, tag="wo2c")
                eng = nc.sync if i % 2 == 0 else nc.gpsimd
                eng.dma_start(out=wo2c[:, :, :w],
                              in_=Wo2[:, :, lo:lo + w].rearrange("c p n -> p c n"))
                ps_o = psb.tile([128, 512], F32, space="PSUM", tag="ps_o")
                for kc in range(4):
                    nc.tensor.matmul(out=ps_o[:, :w], lhsT=s2g[:, kc, :],
                                     rhs=wo2c[:, kc, :w],
                                     start=(kc == 0), stop=False)
                nc.tensor.matmul(out=ps_o[:, :w], lhsT=ones1[:], rhs=bo2_t[:, lo:lo + w],
                                 start=False, stop=True)
                nc.scalar.activation(out=expv[:, lo:lo + w], in_=ps_o[:, :w], func=AF.Exp,
                                     accum_out=vsum[:, i:i + 1])
            vsum_t = wp.tile([128, 1], F32)
            nc.vector.reduce_sum(out=vsum_t[:], in_=vsum[:], axis=mybir.AxisListType.X)
            nc.sync.dma_start(out=vsum_out[:], in_=vsum_t[:])
            nc.sync.dma_start(out=expv_out[:], in_=expv[:])

    _split_multi_waits(nc)
    return nc


def build_program_c():
    """final_cols = expv * (p_gen / S) — the cross-core-normalized scale."""
    nc = bass.Bass()
    expv_in = nc.dram_tensor("expv_in", [B, VL], F32, kind="ExternalInput")
    scale_in = nc.dram_tensor("scale_in", [B, 1], F32, kind="ExternalInput")
    final_cols = nc.dram_tensor("final_cols", [B, VL], F32, kind="ExternalOutput")
    with tile.TileContext(nc) as tc:
        with tc.tile_pool(name="p", bufs=1) as p:
            ev = p.tile([128, VL], F32)
            nc.gpsimd.dma_start(out=ev[:], in_=expv_in[:])
            sc = p.tile([128, 1], F32)
            nc.sync.dma_start(out=sc[:], in_=scale_in[:])
            nc.vector.tensor_scalar_mul(out=ev[:], in0=ev[:], scalar1=sc[:, 0:1])
            nc.sync.dma_start(out=final_cols[:], in_=ev[:])
    _split_multi_waits(nc)
    return nc


# ---------------------------------------------------------------------------
_PROGRAMS: dict = {}


def _get_program(key, builder, *args):
    if key not in _PROGRAMS:
        _PROGRAMS[key] = builder(*args)
    return _PROGRAMS[key]


def _bf(x):
    return np.asarray(x, np.float32).astype(ml_dtypes.bfloat16)


def prep_in_maps(y_t_1, h0, c0, c_t_1, encoder_outputs, encoder_feature, mask_select,
                 enc_batch_extend_vocab, coverage, emb_table, W_c, W_dp, b_dp, v_w,
                 W_xc, b_xc, W_ih, W_hh, b_ih, b_hh, W_pg, b_pg, W_o1, b_o1, W_o2, b_o2):
    to32 = lambda a: np.asarray(a, np.float32)
    y_t_1 = np.asarray(y_t_1)
    h0, c0, c_t_1 = to32(h0), to32(c0), to32(c_t_1)
    encoder_outputs, encoder_feature = to32(encoder_outputs), to32(encoder_feature)
    mask_select, coverage, emb_table = to32(mask_select), to32(coverage), to32(emb_table)
    W_c, W_dp, b_dp, v_w = to32(W_c), to32(W_dp), to32(b_dp), to32(v_w)
    W_xc, b_xc, W_ih, W_hh = to32(W_xc), to32(b_xc), to32(W_ih), to32(W_hh)
    b_ih, b_hh, W_pg, b_pg = to32(b_ih), to32(b_hh), to32(W_pg), to32(b_pg)
    W_o1, b_o1, W_o2, b_o2 = to32(W_o1), to32(b_o1), to32(W_o2), to32(b_o2)

    y_emb = emb_table[y_t_1]                                   # [B, E]
    shared = dict(
        Wxc=np.ascontiguousarray(W_xc.T),
        bxc=b_xc.reshape(E, 1),
        Wih=np.ascontiguousarray(W_ih.T),
        Whh=np.ascontiguousarray(W_hh.T),
        bih2=np.ascontiguousarray(b_ih.reshape(16, 128).T),
        bhh2=np.ascontiguousarray(b_hh.reshape(16, 128).T),
        Wdp=np.ascontiguousarray(W_dp.T),
        bdp2=np.ascontiguousarray(b_dp.reshape(8, 128).T),
        vw2=_bf(v_w[0].reshape(8, 128).T),
        wcrow=_bf(W_c[:, 0].reshape(1, H2)),
        Wpg=np.ascontiguousarray(W_pg[0].reshape(17, 128).T),
        bpg=b_pg.reshape(1, 1),
        Wo1=np.ascontiguousarray(W_o1.T),
        bo12=np.ascontiguousarray(b_o1.reshape(4, 128).T),
    )
    in_maps = []
    for k in range(NCORES):
        bsl = slice(k * BL, (k + 1) * BL)
        m = dict(shared)
        m["xinT"] = np.ascontiguousarray(
            np.concatenate([c_t_1[bsl], y_emb[bsl]], axis=1).T)
        m["h0T"] = np.ascontiguousarray(h0[bsl].T)
        m["c0T"] = np.ascontiguousarray(c0[bsl].T)
        m["encfT"] = _bf(np.ascontiguousarray(encoder_feature[bsl].transpose(0, 2, 1)))
        m["enco"] = _bf(encoder_outputs[bsl])
        m["mask"] = np.ascontiguousarray(mask_select[bsl])
        m["cov"] = np.ascontiguousarray(coverage[bsl])
        m["covrow"] = _bf(coverage[bsl].reshape(1, BL * T))
        in_maps.append(m)

    Wo2T_bf = _bf(np.ascontiguousarray(W_o2.T)).reshape(4, 128, V)
    bo2_bf = _bf(b_o2.reshape(1, V))
    b_maps_wo2 = []
    for k in range(NCORES):
        vsl = slice(k * VL, (k + 1) * VL)
        b_maps_wo2.append(dict(Wo2=np.ascontiguousarray(Wo2T_bf[:, :, vsl]),
                               bo2=np.ascontiguousarray(bo2_bf[:, vsl])))
    use_cov = bool(np.any(coverage != 0.0))
    return in_maps, b_maps_wo2, use_cov, enc_batch_extend_vocab


def _run_spmd(nc, in_maps):
    from concourse.bass_utils import run_bass_kernel_spmd
    return run_bass_kernel_spmd(nc, in_maps, list(range(NCORES))).results


RUN_WALL = {}


def kernel(**inputs):
    import time as _time
    in_maps, b_maps_wo2, use_cov, ebv = prep_in_maps(**inputs)
    nc_a = _get_program(("a", use_cov), build_program_a, use_cov)
    nc_b = _get_program("b", build_program_b)
    nc_c = _get_program("c", build_program_c)

    t0 = _time.time()
    res_a = _run_spmd(nc_a, in_maps)
    RUN_WALL["a"] = _time.time() - t0

    s2_full = np.concatenate([np.asarray(res_a[k]["s2_out"]).T for k in range(NCORES)], 0)
    pg_full = np.concatenate([np.asarray(res_a[k]["pg_out"]) for k in range(NCORES)], 0)
    s2gT = _bf(np.stack([s2_full[:, kc * 128:(kc + 1) * 128].T for kc in range(4)], 0))
    b_in = [dict(s2gT=s2gT, **b_maps_wo2[k]) for k in range(NCORES)]

    t0 = _time.time()
    res_b = _run_spmd(nc_b, b_in)
    RUN_WALL["b"] = _time.time() - t0

    S = np.sum([np.asarray(res_b[k]["vsum_out"]) for k in range(NCORES)], axis=0)  # [B,1]
    scale = (pg_full / S).astype(np.float32)
    c_in = [dict(expv_in=np.asarray(res_b[k]["expv_out"]), scale_in=scale)
            for k in range(NCORES)]

    t0 = _time.time()
    res_c = _run_spmd(nc_c, c_in)
    RUN_WALL["c"] = _time.time() - t0

    final = np.concatenate([np.asarray(res_c[k]["final_cols"]) for k in range(NCORES)], 1)
    h = np.concatenate([np.asarray(res_a[k]["h_out"]).T for k in range(NCORES)], 0)
    c = np.concatenate([np.asarray(res_a[k]["c_out"]).T for k in range(NCORES)], 0)
    c_t = np.concatenate([np.asarray(res_a[k]["ct_out"]).T for k in range(NCORES)], 0)
    attn = np.concatenate([np.asarray(res_a[k]["attn_out"]) for k in range(NCORES)], 0)
    covn = np.concatenate([np.asarray(res_a[k]["covn_out"]) for k in range(NCORES)], 0)

    # pointer scatter-add applied during unshard (host), from device outputs
    rows = np.arange(B)[:, None]
    np.add.at(final, (rows, np.asarray(ebv)), (1.0 - pg_full) * attn)
    return final, h, c, c_t, attn, pg_full, covn
